# revision 12
# baseline (speedup 1.0000x reference)
"""LocalWindowAttention Trainium2 kernel.

Problem: B=8, S=4096, D=1024, H=16 heads, hd=64, window W=64.
  qkv = x @ qkv_w + qkv_b; per-window attention with relative position
  bias; out = attn_out @ proj_w + proj_b.

Sharding: data-parallel over batch — one batch element per NeuronCore
(8 cores), no collectives needed.

Per-core pipeline (S=4096 rows, processed in s-tiles of 512 rows):
  1. x is pre-transposed HOST-side to xT [D, S]; feature-major xt tiles
     stream in with plain wide DMAs (no on-chip transposes).
  2. qT/kT (feature-major) and v (seq-major) via fp16 matmuls vs resident
     qkv_w tiles; fp32 PSUM accumulation over the K=1024 contraction.
  3. Attention per 128-row block (= 2 windows of 64) and per group of 4
     heads:
       scoresT[k,q] = kT.T @ qT      (4 heads -> 4 col-quarters of ONE
                                      [128,512] PSUM tile; the single
                                      reader comes after all 4 writers,
                                      so no PE-write/DVE-read bank race)
       att = exp(scoresT) * expb     (expb = exp(rel_bias) table, fp16;
                                      cross-window entries are exactly 0,
                                      replacing the -1e4 additive mask)
       outT_unnorm[q,hd], denom[q] = att.T @ [v | 1]   (4 heads -> 4
                                      65-col slots of ONE PSUM tile)
       attn_out[q,hd] = outT_unnorm * (1/denom)  (one strided reciprocal
                                      + one broadcast multiply per group)
  4. attn_out PE-transposed per 128-col block; proj matmul; DMA out.

Scale 1/sqrt(hd) is folded into qkv_w's q-columns host-side. qkv_b's
v-part is folded into an effective proj bias host-side (rows of attn sum
to 1). All matmul operands are fp16 (error ~1e-3 vs fp32 reference);
accumulation is always fp32.
"""
import os
import numpy as np

import concourse.bacc as bacc
import concourse.mybir as mybir
from concourse.tile import TileContext
from concourse.bass_utils import run_bass_kernel_spmd
from concourse.masks import make_identity

F16 = mybir.dt.float16
F32 = mybir.dt.float32

B, S, D = 8, 4096, 1024
H, W, HD = 16, 64, 64
NW = S // W              # 64 windows
STILE = 512              # seq rows per pipeline tile
NST = S // STILE         # 8 s-tiles
NBLK = STILE // 128      # 4 row-blocks (window pairs) per s-tile
MASK = -10000.0          # exp() underflows to exactly 0


# Feature switches (env-overridable for testing). Sharing one PSUM tile
# between 4 score-matmul groups crashed the hardware when the groups MIXED
# stationary base partitions (even heads read kt rows 0-63, odd heads rows
# 64-127); grouping heads by row parity keeps each tile's writers uniform.
SC_SHARED = os.environ.get("KSC", "1") == "1"   # 4 score mms -> one PSUM tile
AV_SHARED = os.environ.get("KAV", "1") == "1"   # 4 av mms -> one PSUM tile
BCAST_NORM = os.environ.get("KBC", "1") == "1"  # batched bcast normalize
DMA_TRANSP = os.environ.get("KDT", "1") == "1"  # attn_out transpose via DMA

# head groups of 4, uniform kt/qt row parity within each group
HGS = [(0, 8, 2), (1, 8, 2), (8, 16, 2), (9, 16, 2)]


def _build(n_stiles=NST, with_qkbias=False, with_projbias=False):
    nc = bacc.Bacc()
    s_total = n_stiles * STILE

    xt_ext = nc.declare_dram_parameter("xt16", [D, s_total], F16, isOutput=False)
    w_ext = nc.declare_dram_parameter("qkvw16", [D, 3 * D], F16, isOutput=False)
    pw_ext = nc.declare_dram_parameter("projw16", [D, D], F16, isOutput=False)
    eb_ext = nc.declare_dram_parameter("expb16", [128, H * 128], F16,
                                       isOutput=False)
    out_ext = nc.declare_dram_parameter("out", [s_total, D], F32, isOutput=True)
    if with_qkbias:
        qkb_ext = nc.declare_dram_parameter("qkb", [16, 128, 1], F32,
                                            isOutput=False)
    if with_projbias:
        pbb_ext = nc.declare_dram_parameter("projb_bcast", [128, D], F32,
                                            isOutput=False)

    with TileContext(nc) as tc:
        with (
            tc.tile_pool(name="const", bufs=1) as const,
            tc.tile_pool(name="xtp", bufs=2) as xtp,
            tc.tile_pool(name="qktp", bufs=2) as qktp,
            tc.tile_pool(name="vap", bufs=8) as vap,
            tc.tile_pool(name="arp", bufs=4) as arp,
            tc.tile_pool(name="attp", bufs=4) as attp,
            tc.tile_pool(name="rcp", bufs=4) as rcp,
            tc.tile_pool(name="aout", bufs=2) as aout,
            tc.tile_pool(name="atp", bufs=2) as atp,
            tc.tile_pool(name="outp", bufs=2) as outp,
            tc.tile_pool(name="acc", bufs=2, space="PSUM") as acc,
            tc.tile_pool(name="scps", bufs=3 if DMA_TRANSP else 2,
                         space="PSUM") as scps,
            tc.tile_pool(name="aops", bufs=3 if DMA_TRANSP else 2,
                         space="PSUM") as aops,
            tc.tile_pool(name="tp", bufs=2, space="PSUM") as tp,
        ):
            # ---- resident constants -------------------------------------
            wts = []
            for k in range(8):
                wk = const.tile([128, 3 * D], F16, name=f"wk{k}")
                nc.sync.dma_start(out=wk[:], in_=w_ext[k * 128:(k + 1) * 128, :])
                wts.append(wk)
            pwts = []
            for k in range(8):
                pk = const.tile([128, D], F16, name=f"pk{k}")
                nc.sync.dma_start(out=pk[:], in_=pw_ext[k * 128:(k + 1) * 128, :])
                pwts.append(pk)
            ebt = const.tile([128, H * 128], F16, name="ebt")
            nc.sync.dma_start(out=ebt[:], in_=eb_ext[:])
            ident = const.tile([128, 128], F16, name="ident")
            make_identity(nc, ident)
            if with_qkbias:
                qkb = const.tile([128, 16], F32, name="qkb")
                for m in range(16):
                    nc.sync.dma_start(out=qkb[:, m:m + 1], in_=qkb_ext[m])
            if with_projbias:
                pbb = const.tile([128, D], F32, name="pbb")
                nc.sync.dma_start(out=pbb[:], in_=pbb_ext[:])

            # ---- main loop over s-tiles ---------------------------------
            for st in range(n_stiles):
                s0 = st * STILE

                # stage 1: feature-major xt, straight from the
                # pre-transposed DRAM copy
                xt = xtp.tile([128, 8 * STILE], F16, name="xt")
                for c in range(8):
                    nc.sync.dma_start(
                        out=xt[:, c * STILE:(c + 1) * STILE],
                        in_=xt_ext[c * 128:(c + 1) * 128, s0:s0 + STILE])

                # stage 2a: qT, kT (feature-major, fp16)
                qt = qktp.tile([128, 8 * STILE], F16, name="qt", tag="qt")
                kt = qktp.tile([128, 8 * STILE], F16, name="kt", tag="kt")
                for which, dst in ((0, qt), (1, kt)):
                    for m in range(8):
                        ac = acc.tile([128, STILE], F32, name="ac")
                        col0 = which * D + m * 128
                        for k in range(8):
                            nc.tensor.matmul(
                                ac[:],
                                wts[k][:, col0:col0 + 128],
                                xt[:, k * STILE:(k + 1) * STILE],
                                start=(k == 0), stop=(k == 7))
                        dsl = dst[:, m * STILE:(m + 1) * STILE]
                        if with_qkbias:
                            nc.scalar.activation(
                                dsl, ac[:], mybir.ActivationFunctionType.Identity,
                                bias=qkb[:, which * 8 + m:which * 8 + m + 1])
                        else:
                            nc.scalar.copy(dsl, ac[:])

                # stage 2b: v (seq-major, ones column appended per head)
                vts = []
                for b in range(NBLK):
                    vt = vap.tile([128, H * (HD + 1)], F16, name="vt")
                    vt3 = vt.rearrange("p (h c) -> p h c", c=HD + 1)
                    nc.vector.memset(vt3[:, :, HD:HD + 1], 1.0)
                    for n in range(2):
                        ac = acc.tile([128, STILE], F32, name="ac")
                        for k in range(8):
                            nc.tensor.matmul(
                                ac[:],
                                xt[:, k * STILE + b * 128:k * STILE + b * 128 + 128],
                                wts[k][:, 2 * D + n * 512:2 * D + (n + 1) * 512],
                                start=(k == 0), stop=(k == 7))
                        nc.vector.tensor_copy(
                            vt3[:, n * 8:(n + 1) * 8, 0:HD],
                            ac.rearrange("p (h c) -> p h c", c=HD))
                    vts.append(vt)

                # stage 3: attention per window-pair block, 4 heads at a time
                for p in range(NBLK):
                    ao = aout.tile([128, D], F16, name="ao")
                    ao3f = ao.rearrange("p (h c) -> p h c", c=HD)
                    ebt3 = ebt.rearrange("p (h q) -> p h q", q=128)
                    vt3 = vts[p].rearrange("p (h c) -> p h c", c=HD + 1)
                    for hg in range(4):
                        heads = list(range(*HGS[hg]))
                        r = (heads[0] % 2) * 64
                        araw = arp.tile([128, 512], F16, name="araw")
                        if SC_SHARED:
                            # 4 score matmuls -> col-quarters of one PSUM
                            # tile (uniform stationary base partition); the
                            # only reader (exp) runs after all four
                            scb = scps.tile([128, 512], F32, name="scb")
                            for i, h in enumerate(heads):
                                c0 = (h // 2) * STILE + p * 128
                                nc.tensor.matmul(
                                    scb[:, i * 128:(i + 1) * 128],
                                    kt[r:r + 64, c0:c0 + 128],
                                    qt[r:r + 64, c0:c0 + 128],
                                    start=True, stop=True)
                            nc.scalar.activation(
                                araw[:], scb[:],
                                mybir.ActivationFunctionType.Exp)
                        else:
                            for i, h in enumerate(heads):
                                c0 = (h // 2) * STILE + p * 128
                                sc = scps.tile([128, 128], F32, name="scb")
                                nc.tensor.matmul(
                                    sc[:],
                                    kt[r:r + 64, c0:c0 + 128],
                                    qt[r:r + 64, c0:c0 + 128],
                                    start=True, stop=True)
                                nc.scalar.activation(
                                    araw[:, i * 128:(i + 1) * 128], sc[:],
                                    mybir.ActivationFunctionType.Exp)
                        att = attp.tile([128, 512], F16, name="att")
                        nc.vector.tensor_mul(
                            att[:], araw[:],
                            ebt3[:, HGS[hg][0]:HGS[hg][1]:HGS[hg][2], :])
                        if AV_SHARED:
                            # 4 attention-weighted-V matmuls -> 65-col slots
                            # of one PSUM tile (col 64 of each = denominator)
                            aop4 = aops.tile([128, 4 * (HD + 1)], F32,
                                             name="aop4")
                            a3 = aop4.rearrange("p (i c) -> p i c", c=HD + 1)
                            for i, h in enumerate(heads):
                                nc.tensor.matmul(
                                    aop4[:, i * (HD + 1):(i + 1) * (HD + 1)],
                                    att[:, i * 128:(i + 1) * 128],
                                    vt3[:, h, :],
                                    start=True, stop=True)
                            if BCAST_NORM:
                                rc4 = rcp.tile([128, 4], F32, name="rc4")
                                nc.vector.reciprocal(rc4[:], a3[:, :, HD])
                                ao3 = ao3f[:, HGS[hg][0]:HGS[hg][1]:HGS[hg][2], :]
                                nc.vector.tensor_mul(
                                    ao3, a3[:, :, 0:HD],
                                    rc4.rearrange(
                                        "p (i j) -> p i j", j=1).broadcast_to(
                                        [128, 4, HD]))
                            else:
                                for i, h in enumerate(heads):
                                    rc = rcp.tile([128, 1], F32, name="rc4")
                                    nc.vector.reciprocal(
                                        rc[:], a3[:, i, HD:HD + 1])
                                    nc.vector.tensor_scalar_mul(
                                        ao[:, h * 64:(h + 1) * 64],
                                        a3[:, i, 0:HD], rc[:])
                        else:
                            for i, h in enumerate(heads):
                                aop = aops.tile([128, 128], F32, name="aop4")
                                nc.tensor.matmul(
                                    aop[:, :HD + 1],
                                    att[:, i * 128:(i + 1) * 128],
                                    vt3[:, h, :],
                                    start=True, stop=True)
                                rc = rcp.tile([128, 1], F32, name="rc4")
                                nc.vector.reciprocal(rc[:], aop[:, HD:HD + 1])
                                nc.vector.tensor_scalar_mul(
                                    ao[:, h * 64:(h + 1) * 64],
                                    aop[:, :HD], rc[:])

                    # stage 4: transpose attn_out, proj matmul, store
                    at = atp.tile([128, D], F16, name="at")
                    if DMA_TRANSP:
                        for c in range(8):
                            nc.sync.dma_start_transpose(
                                at[:, c * 128:(c + 1) * 128],
                                ao[:, c * 128:(c + 1) * 128])
                    else:
                        for c in range(8):
                            tpp = tp.tile([128, 128], F16, name="tpp")
                            nc.tensor.transpose(
                                tpp[:], ao[:, c * 128:(c + 1) * 128], ident[:])
                            nc.vector.tensor_copy(
                                at[:, c * 128:(c + 1) * 128], tpp[:])
                    ot = outp.tile([128, D], F32, name="ot")
                    for n in range(2):
                        ac = acc.tile([128, STILE], F32, name="ac")
                        for k in range(8):
                            nc.tensor.matmul(
                                ac[:],
                                at[:, k * 128:(k + 1) * 128],
                                pwts[k][:, n * 512:(n + 1) * 512],
                                start=(k == 0), stop=(k == 7))
                        nc.scalar.copy(ot[:, n * 512:(n + 1) * 512], ac[:])
                    if with_projbias:
                        nc.vector.tensor_add(ot[:], ot[:], pbb[:])
                    nc.sync.dma_start(
                        out=out_ext[s0 + p * 128:s0 + (p + 1) * 128, :],
                        in_=ot[:])

    nc.compile()
    return nc


def _host_prep(x, qkv_w, qkv_b, proj_w, proj_b, rel_bias):
    """Fold scale/biases, cast to fp16, build the exp'd blocked bias table."""
    scale = 1.0 / np.sqrt(HD)
    qkv_w_s = np.asarray(qkv_w, dtype=np.float64).copy()
    qkv_w_s[:, :D] *= scale
    qkv_b = np.asarray(qkv_b, dtype=np.float64)
    qkv_b_s = qkv_b.copy()
    qkv_b_s[:D] *= scale

    # rel-bias expanded to [H, W, W], packed into the transposed,
    # window-pair [128 (k), H*128 (h-major, q)] table, then EXP'd:
    # att = exp(scores) * exp(bias); masked cross-window entries
    # become exactly 0.
    rb = np.asarray(rel_bias, dtype=np.float32)
    coords = np.arange(W)
    rel = coords[:, None] - coords[None, :] + (W - 1)      # [q, k]
    bias_hqk = rb[rel].transpose(2, 0, 1)                  # [H, q, k]
    b2 = np.full((H, 128, 128), MASK, dtype=np.float32)    # [H, k2, q2]
    bias_kq = bias_hqk.transpose(0, 2, 1)                  # [H, k, q]
    b2[:, :64, :64] = bias_kq
    b2[:, 64:, 64:] = bias_kq
    expb16 = np.ascontiguousarray(
        np.exp(b2.transpose(1, 0, 2)).reshape(128, H * 128)).astype(np.float16)

    # v-bias commutes through attention (rows sum to 1) -> fold into proj_b
    proj_b_eff = (qkv_b[2 * D:] @ np.asarray(proj_w, dtype=np.float64)
                  + np.asarray(proj_b, dtype=np.float64))

    shared = {
        "qkvw16": qkv_w_s.astype(np.float16),
        "projw16": np.asarray(proj_w).astype(np.float16),
        "expb16": expb16,
    }
    qk_bias = qkv_b_s[:2 * D]
    with_qkbias = bool(np.any(qk_bias))
    if with_qkbias:
        shared["qkb"] = np.ascontiguousarray(
            qk_bias.reshape(16, 128, 1).astype(np.float32))
    with_projbias = bool(np.any(proj_b_eff))
    if with_projbias:
        shared["projb_bcast"] = np.broadcast_to(
            proj_b_eff.astype(np.float32), (128, D)).copy()
    return shared, with_qkbias, with_projbias


_NC_CACHE = {}


def kernel(x, qkv_w, qkv_b, proj_w, proj_b, rel_bias):
    x = np.asarray(x)
    shared, wqk, wpb = _host_prep(x, qkv_w, qkv_b, proj_w, proj_b, rel_bias)

    key = (wqk, wpb)
    if key not in _NC_CACHE:
        _NC_CACHE[key] = _build(NST, wqk, wpb)
    nc = _NC_CACHE[key]

    # feature-major xT per batch element (seq stays the fast axis on chip)
    xt16 = np.ascontiguousarray(
        x.astype(np.float16).transpose(0, 2, 1))          # [B, D, S]
    in_maps = [dict(shared, xt16=xt16[i]) for i in range(B)]
    res = run_bass_kernel_spmd(nc, in_maps, list(range(B)))
    return np.stack([res.results[i]["out"] for i in range(B)], axis=0)


if __name__ == "__main__":
    rng = np.random.default_rng(0)
    x = rng.standard_normal((B, S, D), dtype=np.float32)
    qkv_w = rng.standard_normal((D, 3 * D), dtype=np.float32) / np.sqrt(D)
    proj_w = rng.standard_normal((D, D), dtype=np.float32) / np.sqrt(D)
    out = kernel(x, qkv_w, np.zeros(3 * D, np.float32), proj_w,
                 np.zeros(D, np.float32),
                 rng.standard_normal((2 * W - 1, H), dtype=np.float32) * 0.02)
    print(out.shape, out.dtype)


# revision 18
# speedup vs baseline: 1.7096x; 1.7096x over previous
"""LocalWindowAttention Trainium2 kernel.

Problem: B=8, S=4096, D=1024, H=16 heads, hd=64, window W=64.
  qkv = x @ qkv_w + qkv_b; per-window attention with relative position
  bias; out = attn_out @ proj_w + proj_b.

Sharding: data-parallel over batch — one batch element per NeuronCore
(8 cores), no collectives needed.

Per-core pipeline (S=4096 rows, processed in s-tiles of 512 rows):
  1. x is pre-transposed HOST-side to xT [D, S]; feature-major xt tiles
     stream in with plain wide DMAs (no on-chip transposes).
  2. qT/kT (feature-major) and v (seq-major) via fp16 matmuls vs resident
     qkv_w tiles; fp32 PSUM accumulation over the K=1024 contraction.
  3. Attention per 128-row block (= 2 windows of 64) and per group of 4
     heads:
       scoresT[k,q] = kT.T @ qT      (4 heads -> 4 col-quarters of ONE
                                      [128,512] PSUM tile; the single
                                      reader comes after all 4 writers,
                                      so no PE-write/DVE-read bank race)
       att = exp(scoresT) * expb     (expb = exp(rel_bias) table, fp16;
                                      cross-window entries are exactly 0,
                                      replacing the -1e4 additive mask)
       outT_unnorm[q,hd], denom[q] = att.T @ [v | 1]   (4 heads -> 4
                                      65-col slots of ONE PSUM tile)
       attn_out[q,hd] = outT_unnorm * (1/denom)  (one strided reciprocal
                                      + one broadcast multiply per group)
  4. attn_out PE-transposed per 128-col block; proj matmul; DMA out.

Scale 1/sqrt(hd) is folded into qkv_w's q-columns host-side. qkv_b's
v-part is folded into an effective proj bias host-side (rows of attn sum
to 1). All matmul operands are fp16 (error ~1e-3 vs fp32 reference);
accumulation is always fp32.
"""
import os
import numpy as np

import concourse.bacc as bacc
import concourse.mybir as mybir
from concourse.tile import TileContext
from concourse.bass_utils import run_bass_kernel_spmd
from concourse.masks import make_identity

F16 = mybir.dt.float16
F32 = mybir.dt.float32

B, S, D = 8, 4096, 1024
H, W, HD = 16, 64, 64
NW = S // W              # 64 windows
STILE = 512              # seq rows per pipeline tile
NST = S // STILE         # 8 s-tiles
NBLK = STILE // 128      # 4 row-blocks (window pairs) per s-tile
MASK = -10000.0          # exp() underflows to exactly 0


# Feature switches (env-overridable for testing). Sharing one PSUM tile
# between 4 score-matmul groups crashed the hardware when the groups MIXED
# stationary base partitions (even heads read kt rows 0-63, odd heads rows
# 64-127); grouping heads by row parity keeps each tile's writers uniform.
SC_SHARED = os.environ.get("KSC", "1") == "1"   # 4 score mms -> one PSUM tile
AV_SHARED = os.environ.get("KAV", "1") == "1"   # 4 av mms -> one PSUM tile
BCAST_NORM = os.environ.get("KBC", "1") == "1"  # batched bcast normalize
DMA_TRANSP = os.environ.get("KDT", "0") == "1"  # attn_out transpose via DMA
                                                # (measured: WORSE — xbar-mode
                                                # switches serialize the DMA
                                                # stream, 732us -> 1057us)
FEAT_MAJOR = os.environ.get("KFM", "1") == "1"  # feature-major av: out[f,q],
                                                # denom in row 64, no attn_out
                                                # transpose stage at all

# head groups of 4, uniform kt/qt row parity within each group
HGS = [(0, 8, 2), (1, 8, 2), (8, 16, 2), (9, 16, 2)]


def _build(n_stiles=NST, with_qkbias=False, with_projbias=False):
    nc = bacc.Bacc()
    s_total = n_stiles * STILE

    xt_ext = nc.declare_dram_parameter("xt16", [D, s_total], F16, isOutput=False)
    w_ext = nc.declare_dram_parameter("qkvw16", [D, 3 * D], F16, isOutput=False)
    pw_ext = nc.declare_dram_parameter("projw16", [D, D], F16, isOutput=False)
    eb_ext = nc.declare_dram_parameter("expb16", [128, H * 128], F16,
                                       isOutput=False)
    out_ext = nc.declare_dram_parameter("out", [s_total, D], F32, isOutput=True)
    if with_qkbias:
        qkb_ext = nc.declare_dram_parameter("qkb", [16, 128, 1], F32,
                                            isOutput=False)
    if with_projbias:
        pbb_ext = nc.declare_dram_parameter("projb_bcast", [128, D], F32,
                                            isOutput=False)

    with TileContext(nc) as tc:
        with (
            tc.tile_pool(name="const", bufs=1) as const,
            tc.tile_pool(name="xtp", bufs=2) as xtp,
            tc.tile_pool(name="qktp", bufs=2) as qktp,
            tc.tile_pool(name="vap", bufs=8) as vap,
            tc.tile_pool(name="arp", bufs=4) as arp,
            tc.tile_pool(name="attp", bufs=4) as attp,
            tc.tile_pool(name="rcp", bufs=4) as rcp,
            tc.tile_pool(name="aout", bufs=2) as aout,
            tc.tile_pool(name="atp", bufs=2) as atp,
            tc.tile_pool(name="outp", bufs=2) as outp,
            tc.tile_pool(name="acc", bufs=2, space="PSUM") as acc,
            tc.tile_pool(name="scps", bufs=3 if DMA_TRANSP else 2,
                         space="PSUM") as scps,
            tc.tile_pool(name="aops", bufs=3 if DMA_TRANSP else 2,
                         space="PSUM") as aops,
            tc.tile_pool(name="bcps", bufs=2, space="PSUM") as bcps,
            tc.tile_pool(name="tp", bufs=2, space="PSUM") as tp,
        ):
            # ---- resident constants -------------------------------------
            wts = []
            for k in range(8):
                wk = const.tile([128, 3 * D], F16, name=f"wk{k}")
                nc.sync.dma_start(out=wk[:], in_=w_ext[k * 128:(k + 1) * 128, :])
                wts.append(wk)
            pwts = []
            for k in range(8):
                pk = const.tile([128, D], F16, name=f"pk{k}")
                nc.sync.dma_start(out=pk[:], in_=pw_ext[k * 128:(k + 1) * 128, :])
                pwts.append(pk)
            ebt = const.tile([128, H * 128], F16, name="ebt")
            nc.sync.dma_start(out=ebt[:], in_=eb_ext[:])
            ident = const.tile([128, 128], F16, name="ident")
            make_identity(nc, ident)
            onesr = const.tile([1, 64], F16, name="onesr")
            nc.vector.memset(onesr[:], 1.0)
            if with_qkbias:
                qkb = const.tile([128, 16], F32, name="qkb")
                for m in range(16):
                    nc.sync.dma_start(out=qkb[:, m:m + 1], in_=qkb_ext[m])
            if with_projbias:
                pbb = const.tile([128, D], F32, name="pbb")
                nc.sync.dma_start(out=pbb[:], in_=pbb_ext[:])

            # ---- main loop over s-tiles ---------------------------------
            for st in range(n_stiles):
                s0 = st * STILE

                # stage 1: feature-major xt, straight from the
                # pre-transposed DRAM copy
                xt = xtp.tile([128, 8 * STILE], F16, name="xt")
                for c in range(8):
                    nc.sync.dma_start(
                        out=xt[:, c * STILE:(c + 1) * STILE],
                        in_=xt_ext[c * 128:(c + 1) * 128, s0:s0 + STILE])

                # stage 2a: qT, kT (feature-major, fp16)
                qt = qktp.tile([128, 8 * STILE], F16, name="qt", tag="qt")
                kt = qktp.tile([128, 8 * STILE], F16, name="kt", tag="kt")
                for which, dst in ((0, qt), (1, kt)):
                    for m in range(8):
                        ac = acc.tile([128, STILE], F32, name="ac")
                        col0 = which * D + m * 128
                        for k in range(8):
                            nc.tensor.matmul(
                                ac[:],
                                wts[k][:, col0:col0 + 128],
                                xt[:, k * STILE:(k + 1) * STILE],
                                start=(k == 0), stop=(k == 7))
                        dsl = dst[:, m * STILE:(m + 1) * STILE]
                        if with_qkbias:
                            nc.scalar.activation(
                                dsl, ac[:], mybir.ActivationFunctionType.Identity,
                                bias=qkb[:, which * 8 + m:which * 8 + m + 1])
                        else:
                            nc.scalar.copy(dsl, ac[:])

                # stage 2b: v (seq-major, ones column appended per head)
                vts = []
                for b in range(NBLK):
                    vt = vap.tile([128, H * (HD + 1)], F16, name="vt")
                    vt3 = vt.rearrange("p (h c) -> p h c", c=HD + 1)
                    nc.vector.memset(vt3[:, :, HD:HD + 1], 1.0)
                    for n in range(2):
                        ac = acc.tile([128, STILE], F32, name="ac")
                        for k in range(8):
                            nc.tensor.matmul(
                                ac[:],
                                xt[:, k * STILE + b * 128:k * STILE + b * 128 + 128],
                                wts[k][:, 2 * D + n * 512:2 * D + (n + 1) * 512],
                                start=(k == 0), stop=(k == 7))
                        nc.vector.tensor_copy(
                            vt3[:, n * 8:(n + 1) * 8, 0:HD],
                            ac.rearrange("p (h c) -> p h c", c=HD))
                    vts.append(vt)

                # stage 3: attention per window-pair block, 4 heads at a time
                for p in range(NBLK):
                    ebt3 = ebt.rearrange("p (h q) -> p h q", q=128)
                    vt3 = vts[p].rearrange("p (h c) -> p h c", c=HD + 1)
                    if FEAT_MAJOR:
                        # at[f-in-block, 8 blocks x 128 q] is written
                        # directly by the normalization multiplies; each
                        # parity head-group covers one [64, 512] quadrant
                        at = atp.tile([128, D], F16, name="at")
                        for hg in range(4):
                            heads = list(range(*HGS[hg]))
                            r = (heads[0] % 2) * 64
                            araw = arp.tile([128, 512], F16, name="araw")
                            scb = scps.tile([128, 512], F32, name="scb")
                            for i, h in enumerate(heads):
                                c0 = (h // 2) * STILE + p * 128
                                nc.tensor.matmul(
                                    scb[:, i * 128:(i + 1) * 128],
                                    kt[r:r + 64, c0:c0 + 128],
                                    qt[r:r + 64, c0:c0 + 128],
                                    start=True, stop=True)
                            nc.scalar.activation(
                                araw[:], scb[:],
                                mybir.ActivationFunctionType.Exp)
                            att = attp.tile([128, 512], F16, name="att")
                            nc.vector.tensor_mul(
                                att[:], araw[:],
                                ebt3[:, HGS[hg][0]:HGS[hg][1]:HGS[hg][2], :])
                            # v-stationary av: out [65, 128q] per head; all
                            # four share one PSUM bank at base partition 0;
                            # row 64 = softmax denominators
                            av4 = aops.tile([65, 512], F32, name="aop4")
                            for i, h in enumerate(heads):
                                nc.tensor.matmul(
                                    av4[:, i * 128:(i + 1) * 128],
                                    vt3[:, h, :],
                                    att[:, i * 128:(i + 1) * 128],
                                    start=True, stop=True)
                            rcr = rcp.tile([1, 512], F16, name="rcr")
                            with nc.allow_low_precision(
                                    reason="softmax reciprocal fp16"):
                                nc.vector.reciprocal(rcr[:], av4[64:65, :])
                            # DVE can't read stride-0 partition APs, so
                            # materialize 1 (x) rc with a K=1 outer-product
                            # matmul; DVE also can't read two PSUM inputs,
                            # so the unnormalized rows drain to SBUF via the
                            # scalar engine (parallel to the rc chain)
                            rcb = bcps.tile([64, 512], F32, name="rcb")
                            nc.tensor.matmul(rcb[:], onesr[:], rcr[:],
                                             start=True, stop=True)
                            avs = arp.tile([64, 512], F16, name="avs",
                                           tag="avs")
                            nc.scalar.copy(avs[:], av4[0:64, :])
                            cb = 0 if hg < 2 else 512
                            nc.vector.tensor_mul(
                                at[r:r + 64, cb:cb + 512],
                                avs[:], rcb[:])
                        ot = outp.tile([128, D], F32, name="ot")
                        for n in range(2):
                            ac = acc.tile([128, STILE], F32, name="ac")
                            for k in range(8):
                                nc.tensor.matmul(
                                    ac[:],
                                    at[:, k * 128:(k + 1) * 128],
                                    pwts[k][:, n * 512:(n + 1) * 512],
                                    start=(k == 0), stop=(k == 7))
                            nc.scalar.copy(ot[:, n * 512:(n + 1) * 512], ac[:])
                        if with_projbias:
                            nc.vector.tensor_add(ot[:], ot[:], pbb[:])
                        nc.sync.dma_start(
                            out=out_ext[s0 + p * 128:s0 + (p + 1) * 128, :],
                            in_=ot[:])
                        continue
                    ao = aout.tile([128, D], F16, name="ao")
                    ao3f = ao.rearrange("p (h c) -> p h c", c=HD)
                    for hg in range(4):
                        heads = list(range(*HGS[hg]))
                        r = (heads[0] % 2) * 64
                        araw = arp.tile([128, 512], F16, name="araw")
                        if SC_SHARED:
                            # 4 score matmuls -> col-quarters of one PSUM
                            # tile (uniform stationary base partition); the
                            # only reader (exp) runs after all four
                            scb = scps.tile([128, 512], F32, name="scb")
                            for i, h in enumerate(heads):
                                c0 = (h // 2) * STILE + p * 128
                                nc.tensor.matmul(
                                    scb[:, i * 128:(i + 1) * 128],
                                    kt[r:r + 64, c0:c0 + 128],
                                    qt[r:r + 64, c0:c0 + 128],
                                    start=True, stop=True)
                            nc.scalar.activation(
                                araw[:], scb[:],
                                mybir.ActivationFunctionType.Exp)
                        else:
                            for i, h in enumerate(heads):
                                c0 = (h // 2) * STILE + p * 128
                                sc = scps.tile([128, 128], F32, name="scb")
                                nc.tensor.matmul(
                                    sc[:],
                                    kt[r:r + 64, c0:c0 + 128],
                                    qt[r:r + 64, c0:c0 + 128],
                                    start=True, stop=True)
                                nc.scalar.activation(
                                    araw[:, i * 128:(i + 1) * 128], sc[:],
                                    mybir.ActivationFunctionType.Exp)
                        att = attp.tile([128, 512], F16, name="att")
                        nc.vector.tensor_mul(
                            att[:], araw[:],
                            ebt3[:, HGS[hg][0]:HGS[hg][1]:HGS[hg][2], :])
                        if AV_SHARED:
                            # 4 attention-weighted-V matmuls -> 65-col slots
                            # of one PSUM tile (col 64 of each = denominator)
                            aop4 = aops.tile([128, 4 * (HD + 1)], F32,
                                             name="aop4")
                            a3 = aop4.rearrange("p (i c) -> p i c", c=HD + 1)
                            for i, h in enumerate(heads):
                                nc.tensor.matmul(
                                    aop4[:, i * (HD + 1):(i + 1) * (HD + 1)],
                                    att[:, i * 128:(i + 1) * 128],
                                    vt3[:, h, :],
                                    start=True, stop=True)
                            if BCAST_NORM:
                                rc4 = rcp.tile([128, 4], F32, name="rc4")
                                nc.vector.reciprocal(rc4[:], a3[:, :, HD])
                                ao3 = ao3f[:, HGS[hg][0]:HGS[hg][1]:HGS[hg][2], :]
                                nc.vector.tensor_mul(
                                    ao3, a3[:, :, 0:HD],
                                    rc4.rearrange(
                                        "p (i j) -> p i j", j=1).broadcast_to(
                                        [128, 4, HD]))
                            else:
                                for i, h in enumerate(heads):
                                    rc = rcp.tile([128, 1], F32, name="rc4")
                                    nc.vector.reciprocal(
                                        rc[:], a3[:, i, HD:HD + 1])
                                    nc.vector.tensor_scalar_mul(
                                        ao[:, h * 64:(h + 1) * 64],
                                        a3[:, i, 0:HD], rc[:])
                        else:
                            for i, h in enumerate(heads):
                                aop = aops.tile([128, 128], F32, name="aop4")
                                nc.tensor.matmul(
                                    aop[:, :HD + 1],
                                    att[:, i * 128:(i + 1) * 128],
                                    vt3[:, h, :],
                                    start=True, stop=True)
                                rc = rcp.tile([128, 1], F32, name="rc4")
                                nc.vector.reciprocal(rc[:], aop[:, HD:HD + 1])
                                nc.vector.tensor_scalar_mul(
                                    ao[:, h * 64:(h + 1) * 64],
                                    aop[:, :HD], rc[:])

                    # stage 4: transpose attn_out, proj matmul, store
                    at = atp.tile([128, D], F16, name="at")
                    if DMA_TRANSP:
                        for c in range(8):
                            nc.sync.dma_start_transpose(
                                at[:, c * 128:(c + 1) * 128],
                                ao[:, c * 128:(c + 1) * 128])
                    else:
                        for c in range(8):
                            tpp = tp.tile([128, 128], F16, name="tpp")
                            nc.tensor.transpose(
                                tpp[:], ao[:, c * 128:(c + 1) * 128], ident[:])
                            nc.vector.tensor_copy(
                                at[:, c * 128:(c + 1) * 128], tpp[:])
                    ot = outp.tile([128, D], F32, name="ot")
                    for n in range(2):
                        ac = acc.tile([128, STILE], F32, name="ac")
                        for k in range(8):
                            nc.tensor.matmul(
                                ac[:],
                                at[:, k * 128:(k + 1) * 128],
                                pwts[k][:, n * 512:(n + 1) * 512],
                                start=(k == 0), stop=(k == 7))
                        nc.scalar.copy(ot[:, n * 512:(n + 1) * 512], ac[:])
                    if with_projbias:
                        nc.vector.tensor_add(ot[:], ot[:], pbb[:])
                    nc.sync.dma_start(
                        out=out_ext[s0 + p * 128:s0 + (p + 1) * 128, :],
                        in_=ot[:])

    nc.compile()
    return nc


def _host_prep(x, qkv_w, qkv_b, proj_w, proj_b, rel_bias):
    """Fold scale/biases, cast to fp16, build the exp'd blocked bias table."""
    scale = 1.0 / np.sqrt(HD)
    qkv_w_s = np.asarray(qkv_w, dtype=np.float64).copy()
    qkv_w_s[:, :D] *= scale
    qkv_b = np.asarray(qkv_b, dtype=np.float64)
    qkv_b_s = qkv_b.copy()
    qkv_b_s[:D] *= scale

    # rel-bias expanded to [H, W, W], packed into the transposed,
    # window-pair [128 (k), H*128 (h-major, q)] table, then EXP'd:
    # att = exp(scores) * exp(bias); masked cross-window entries
    # become exactly 0.
    rb = np.asarray(rel_bias, dtype=np.float32)
    coords = np.arange(W)
    rel = coords[:, None] - coords[None, :] + (W - 1)      # [q, k]
    bias_hqk = rb[rel].transpose(2, 0, 1)                  # [H, q, k]
    b2 = np.full((H, 128, 128), MASK, dtype=np.float32)    # [H, k2, q2]
    bias_kq = bias_hqk.transpose(0, 2, 1)                  # [H, k, q]
    b2[:, :64, :64] = bias_kq
    b2[:, 64:, 64:] = bias_kq
    expb16 = np.ascontiguousarray(
        np.exp(b2.transpose(1, 0, 2)).reshape(128, H * 128)).astype(np.float16)

    # v-bias commutes through attention (rows sum to 1) -> fold into proj_b
    proj_b_eff = (qkv_b[2 * D:] @ np.asarray(proj_w, dtype=np.float64)
                  + np.asarray(proj_b, dtype=np.float64))

    shared = {
        "qkvw16": qkv_w_s.astype(np.float16),
        "projw16": np.asarray(proj_w).astype(np.float16),
        "expb16": expb16,
    }
    qk_bias = qkv_b_s[:2 * D]
    with_qkbias = bool(np.any(qk_bias))
    if with_qkbias:
        shared["qkb"] = np.ascontiguousarray(
            qk_bias.reshape(16, 128, 1).astype(np.float32))
    with_projbias = bool(np.any(proj_b_eff))
    if with_projbias:
        shared["projb_bcast"] = np.broadcast_to(
            proj_b_eff.astype(np.float32), (128, D)).copy()
    return shared, with_qkbias, with_projbias


_NC_CACHE = {}


def kernel(x, qkv_w, qkv_b, proj_w, proj_b, rel_bias):
    x = np.asarray(x)
    shared, wqk, wpb = _host_prep(x, qkv_w, qkv_b, proj_w, proj_b, rel_bias)

    key = (wqk, wpb)
    if key not in _NC_CACHE:
        _NC_CACHE[key] = _build(NST, wqk, wpb)
    nc = _NC_CACHE[key]

    # feature-major xT per batch element (seq stays the fast axis on chip)
    xt16 = np.ascontiguousarray(
        x.astype(np.float16).transpose(0, 2, 1))          # [B, D, S]
    in_maps = [dict(shared, xt16=xt16[i]) for i in range(B)]
    res = run_bass_kernel_spmd(nc, in_maps, list(range(B)))
    return np.stack([res.results[i]["out"] for i in range(B)], axis=0)


if __name__ == "__main__":
    rng = np.random.default_rng(0)
    x = rng.standard_normal((B, S, D), dtype=np.float32)
    qkv_w = rng.standard_normal((D, 3 * D), dtype=np.float32) / np.sqrt(D)
    proj_w = rng.standard_normal((D, D), dtype=np.float32) / np.sqrt(D)
    out = kernel(x, qkv_w, np.zeros(3 * D, np.float32), proj_w,
                 np.zeros(D, np.float32),
                 rng.standard_normal((2 * W - 1, H), dtype=np.float32) * 0.02)
    print(out.shape, out.dtype)


# revision 21
# speedup vs baseline: 1.7553x; 1.0267x over previous
"""LocalWindowAttention Trainium2 kernel.

Problem: B=8, S=4096, D=1024, H=16 heads, hd=64, window W=64.
  qkv = x @ qkv_w + qkv_b; per-window attention with relative position
  bias; out = attn_out @ proj_w + proj_b.

Sharding: data-parallel over batch — one batch element per NeuronCore
(8 cores), no collectives needed.

Per-core pipeline (S=4096 rows, processed in s-tiles of 512 rows):
  1. x is pre-transposed HOST-side to xT [D, S]; feature-major xt tiles
     stream in with plain wide DMAs (no on-chip transposes).
  2. qT/kT (feature-major) and v (seq-major) via fp16 matmuls vs resident
     qkv_w tiles; fp32 PSUM accumulation over the K=1024 contraction.
  3. Attention per 128-row block (= 2 windows of 64) and per group of 4
     heads:
       scoresT[k,q] = kT.T @ qT      (4 heads -> 4 col-quarters of ONE
                                      [128,512] PSUM tile; the single
                                      reader comes after all 4 writers,
                                      so no PE-write/DVE-read bank race)
       att = exp(scoresT) * expb     (expb = exp(rel_bias) table, fp16;
                                      cross-window entries are exactly 0,
                                      replacing the -1e4 additive mask)
       outT_unnorm[q,hd], denom[q] = att.T @ [v | 1]   (4 heads -> 4
                                      65-col slots of ONE PSUM tile)
       attn_out[q,hd] = outT_unnorm * (1/denom)  (one strided reciprocal
                                      + one broadcast multiply per group)
  4. attn_out PE-transposed per 128-col block; proj matmul; DMA out.

Scale 1/sqrt(hd) is folded into qkv_w's q-columns host-side. qkv_b's
v-part is folded into an effective proj bias host-side (rows of attn sum
to 1). All matmul operands are fp16 (error ~1e-3 vs fp32 reference);
accumulation is always fp32.
"""
import os
import numpy as np

import concourse.bacc as bacc
import concourse.mybir as mybir
from concourse.tile import TileContext
from concourse.bass_utils import run_bass_kernel_spmd
from concourse.masks import make_identity

F16 = mybir.dt.float16
F32 = mybir.dt.float32

B, S, D = 8, 4096, 1024
H, W, HD = 16, 64, 64
NW = S // W              # 64 windows
STILE = 512              # seq rows per pipeline tile
NST = S // STILE         # 8 s-tiles
NBLK = STILE // 128      # 4 row-blocks (window pairs) per s-tile
MASK = -10000.0          # exp() underflows to exactly 0


# Feature switches (env-overridable for testing). Sharing one PSUM tile
# between 4 score-matmul groups crashed the hardware when the groups MIXED
# stationary base partitions (even heads read kt rows 0-63, odd heads rows
# 64-127); grouping heads by row parity keeps each tile's writers uniform.
SC_SHARED = os.environ.get("KSC", "1") == "1"   # 4 score mms -> one PSUM tile
AV_SHARED = os.environ.get("KAV", "1") == "1"   # 4 av mms -> one PSUM tile
BCAST_NORM = os.environ.get("KBC", "1") == "1"  # batched bcast normalize
DMA_TRANSP = os.environ.get("KDT", "0") == "1"  # attn_out transpose via DMA
                                                # (measured: WORSE — xbar-mode
                                                # switches serialize the DMA
                                                # stream, 732us -> 1057us)
FEAT_MAJOR = os.environ.get("KFM", "0") == "1"  # feature-major av: out[f,q],
                                                # denom in row 64, no attn_out
                                                # transpose stage (measured:
                                                # WORSE — the [1,512] one-
                                                # partition DVE reciprocal on
                                                # the critical path costs
                                                # 3.3us per head-group)

# head groups of 4, uniform kt/qt row parity within each group
HGS = [(0, 8, 2), (1, 8, 2), (8, 16, 2), (9, 16, 2)]


def _build(n_stiles=NST, with_qkbias=False, with_projbias=False):
    nc = bacc.Bacc()
    s_total = n_stiles * STILE

    xt_ext = nc.declare_dram_parameter("xt16", [D, s_total], F16, isOutput=False)
    w_ext = nc.declare_dram_parameter("qkvw16", [D, 3 * D], F16, isOutput=False)
    pw_ext = nc.declare_dram_parameter("projw16", [D, D], F16, isOutput=False)
    eb_ext = nc.declare_dram_parameter("expb16", [128, H * 128], F16,
                                       isOutput=False)
    out_ext = nc.declare_dram_parameter("out", [s_total, D], F32, isOutput=True)
    if with_qkbias:
        qkb_ext = nc.declare_dram_parameter("qkb", [16, 128, 1], F32,
                                            isOutput=False)
    if with_projbias:
        pbb_ext = nc.declare_dram_parameter("projb_bcast", [128, D], F32,
                                            isOutput=False)

    with TileContext(nc) as tc:
        with (
            tc.tile_pool(name="const", bufs=1) as const,
            tc.tile_pool(name="xtp", bufs=2) as xtp,
            tc.tile_pool(name="qktp", bufs=2) as qktp,
            tc.tile_pool(name="vap", bufs=8) as vap,
            tc.tile_pool(name="arp", bufs=4) as arp,
            tc.tile_pool(name="attp", bufs=4) as attp,
            tc.tile_pool(name="rcp", bufs=4) as rcp,
            tc.tile_pool(name="aout", bufs=2) as aout,
            tc.tile_pool(name="atp", bufs=2) as atp,
            tc.tile_pool(name="outp", bufs=2) as outp,
            tc.tile_pool(name="acc", bufs=2, space="PSUM") as acc,
            tc.tile_pool(name="scps", bufs=3, space="PSUM") as scps,
            tc.tile_pool(name="aops", bufs=2, space="PSUM") as aops,
            tc.tile_pool(name="bcps", bufs=2, space="PSUM") as bcps,
            tc.tile_pool(name="tp", bufs=1, space="PSUM") as tp,
        ):
            # ---- resident constants -------------------------------------
            wts = []
            for k in range(8):
                wk = const.tile([128, 3 * D], F16, name=f"wk{k}")
                nc.sync.dma_start(out=wk[:], in_=w_ext[k * 128:(k + 1) * 128, :])
                wts.append(wk)
            pwts = []
            for k in range(8):
                pk = const.tile([128, D], F16, name=f"pk{k}")
                nc.sync.dma_start(out=pk[:], in_=pw_ext[k * 128:(k + 1) * 128, :])
                pwts.append(pk)
            ebt = const.tile([128, H * 128], F16, name="ebt")
            nc.sync.dma_start(out=ebt[:], in_=eb_ext[:])
            ident = const.tile([128, 128], F16, name="ident")
            make_identity(nc, ident)
            onesr = const.tile([1, 64], F16, name="onesr")
            nc.vector.memset(onesr[:], 1.0)
            if with_qkbias:
                qkb = const.tile([128, 16], F32, name="qkb")
                for m in range(16):
                    nc.sync.dma_start(out=qkb[:, m:m + 1], in_=qkb_ext[m])
            if with_projbias:
                pbb = const.tile([128, D], F32, name="pbb")
                nc.sync.dma_start(out=pbb[:], in_=pbb_ext[:])

            # ---- main loop over s-tiles ---------------------------------
            for st in range(n_stiles):
                s0 = st * STILE

                # stage 1: feature-major xt, straight from the
                # pre-transposed DRAM copy
                xt = xtp.tile([128, 8 * STILE], F16, name="xt")
                for c in range(8):
                    nc.sync.dma_start(
                        out=xt[:, c * STILE:(c + 1) * STILE],
                        in_=xt_ext[c * 128:(c + 1) * 128, s0:s0 + STILE])

                # stage 2a: qT, kT (feature-major, fp16)
                qt = qktp.tile([128, 8 * STILE], F16, name="qt", tag="qt")
                kt = qktp.tile([128, 8 * STILE], F16, name="kt", tag="kt")
                # m-blocks interleaved q,k so attention head-groups (which
                # need q AND k of m 0-3 first) unblock after 8 drains, not 12
                for m in range(8):
                    for which, dst in ((0, qt), (1, kt)):
                        ac = acc.tile([128, STILE], F32, name="ac")
                        col0 = which * D + m * 128
                        for k in range(8):
                            nc.tensor.matmul(
                                ac[:],
                                wts[k][:, col0:col0 + 128],
                                xt[:, k * STILE:(k + 1) * STILE],
                                start=(k == 0), stop=(k == 7))
                        dsl = dst[:, m * STILE:(m + 1) * STILE]
                        if with_qkbias:
                            nc.scalar.activation(
                                dsl, ac[:], mybir.ActivationFunctionType.Identity,
                                bias=qkb[:, which * 8 + m:which * 8 + m + 1])
                        else:
                            nc.scalar.copy(dsl, ac[:])

                # stage 2b: v (seq-major, ones column appended per head)
                vts = []
                for b in range(NBLK):
                    vt = vap.tile([128, H * (HD + 1)], F16, name="vt")
                    vt3 = vt.rearrange("p (h c) -> p h c", c=HD + 1)
                    nc.vector.memset(vt3[:, :, HD:HD + 1], 1.0)
                    for n in range(2):
                        ac = acc.tile([128, STILE], F32, name="ac")
                        for k in range(8):
                            nc.tensor.matmul(
                                ac[:],
                                xt[:, k * STILE + b * 128:k * STILE + b * 128 + 128],
                                wts[k][:, 2 * D + n * 512:2 * D + (n + 1) * 512],
                                start=(k == 0), stop=(k == 7))
                        nc.vector.tensor_copy(
                            vt3[:, n * 8:(n + 1) * 8, 0:HD],
                            ac.rearrange("p (h c) -> p h c", c=HD))
                    vts.append(vt)

                # stage 3: attention per window-pair block, 4 heads at a time
                for p in range(NBLK):
                    ebt3 = ebt.rearrange("p (h q) -> p h q", q=128)
                    vt3 = vts[p].rearrange("p (h c) -> p h c", c=HD + 1)
                    if FEAT_MAJOR:
                        # at[f-in-block, 8 blocks x 128 q] is written
                        # directly by the normalization multiplies; each
                        # parity head-group covers one [64, 512] quadrant
                        at = atp.tile([128, D], F16, name="at")
                        for hg in range(4):
                            heads = list(range(*HGS[hg]))
                            r = (heads[0] % 2) * 64
                            araw = arp.tile([128, 512], F16, name="araw")
                            scb = scps.tile([128, 512], F32, name="scb")
                            for i, h in enumerate(heads):
                                c0 = (h // 2) * STILE + p * 128
                                nc.tensor.matmul(
                                    scb[:, i * 128:(i + 1) * 128],
                                    kt[r:r + 64, c0:c0 + 128],
                                    qt[r:r + 64, c0:c0 + 128],
                                    start=True, stop=True)
                            nc.scalar.activation(
                                araw[:], scb[:],
                                mybir.ActivationFunctionType.Exp)
                            att = attp.tile([128, 512], F16, name="att")
                            nc.vector.tensor_mul(
                                att[:], araw[:],
                                ebt3[:, HGS[hg][0]:HGS[hg][1]:HGS[hg][2], :])
                            # v-stationary av: out [65, 128q] per head; all
                            # four share one PSUM bank at base partition 0;
                            # row 64 = softmax denominators
                            av4 = aops.tile([65, 512], F32, name="aop4")
                            for i, h in enumerate(heads):
                                nc.tensor.matmul(
                                    av4[:, i * 128:(i + 1) * 128],
                                    vt3[:, h, :],
                                    att[:, i * 128:(i + 1) * 128],
                                    start=True, stop=True)
                            rcr = rcp.tile([1, 512], F16, name="rcr")
                            with nc.allow_low_precision(
                                    reason="softmax reciprocal fp16"):
                                nc.vector.reciprocal(rcr[:], av4[64:65, :])
                            # DVE can't read stride-0 partition APs, so
                            # materialize 1 (x) rc with a K=1 outer-product
                            # matmul; DVE also can't read two PSUM inputs,
                            # so the unnormalized rows drain to SBUF via the
                            # scalar engine (parallel to the rc chain)
                            rcb = bcps.tile([64, 512], F32, name="rcb")
                            nc.tensor.matmul(rcb[:], onesr[:], rcr[:],
                                             start=True, stop=True)
                            avs = arp.tile([64, 512], F16, name="avs",
                                           tag="avs")
                            nc.scalar.copy(avs[:], av4[0:64, :])
                            cb = 0 if hg < 2 else 512
                            nc.vector.tensor_mul(
                                at[r:r + 64, cb:cb + 512],
                                avs[:], rcb[:])
                        ot = outp.tile([128, D], F32, name="ot")
                        for n in range(2):
                            ac = acc.tile([128, STILE], F32, name="ac")
                            for k in range(8):
                                nc.tensor.matmul(
                                    ac[:],
                                    at[:, k * 128:(k + 1) * 128],
                                    pwts[k][:, n * 512:(n + 1) * 512],
                                    start=(k == 0), stop=(k == 7))
                            nc.scalar.copy(ot[:, n * 512:(n + 1) * 512], ac[:])
                        if with_projbias:
                            nc.vector.tensor_add(ot[:], ot[:], pbb[:])
                        nc.sync.dma_start(
                            out=out_ext[s0 + p * 128:s0 + (p + 1) * 128, :],
                            in_=ot[:])
                        continue
                    ao = aout.tile([128, D], F16, name="ao")
                    ao3f = ao.rearrange("p (h c) -> p h c", c=HD)
                    for hg in range(4):
                        heads = list(range(*HGS[hg]))
                        r = (heads[0] % 2) * 64
                        araw = arp.tile([128, 512], F16, name="araw")
                        if SC_SHARED:
                            # 4 score matmuls -> col-quarters of one PSUM
                            # tile (uniform stationary base partition); the
                            # only reader (exp) runs after all four
                            scb = scps.tile([128, 512], F32, name="scb")
                            for i, h in enumerate(heads):
                                c0 = (h // 2) * STILE + p * 128
                                nc.tensor.matmul(
                                    scb[:, i * 128:(i + 1) * 128],
                                    kt[r:r + 64, c0:c0 + 128],
                                    qt[r:r + 64, c0:c0 + 128],
                                    start=True, stop=True)
                            nc.scalar.activation(
                                araw[:], scb[:],
                                mybir.ActivationFunctionType.Exp)
                        else:
                            for i, h in enumerate(heads):
                                c0 = (h // 2) * STILE + p * 128
                                sc = scps.tile([128, 128], F32, name="scb")
                                nc.tensor.matmul(
                                    sc[:],
                                    kt[r:r + 64, c0:c0 + 128],
                                    qt[r:r + 64, c0:c0 + 128],
                                    start=True, stop=True)
                                nc.scalar.activation(
                                    araw[:, i * 128:(i + 1) * 128], sc[:],
                                    mybir.ActivationFunctionType.Exp)
                        att = attp.tile([128, 512], F16, name="att")
                        nc.vector.tensor_mul(
                            att[:], araw[:],
                            ebt3[:, HGS[hg][0]:HGS[hg][1]:HGS[hg][2], :])
                        if AV_SHARED:
                            # 4 attention-weighted-V matmuls -> 65-col slots
                            # of one PSUM tile (col 64 of each = denominator)
                            aop4 = aops.tile([128, 4 * (HD + 1)], F32,
                                             name="aop4")
                            a3 = aop4.rearrange("p (i c) -> p i c", c=HD + 1)
                            for i, h in enumerate(heads):
                                nc.tensor.matmul(
                                    aop4[:, i * (HD + 1):(i + 1) * (HD + 1)],
                                    att[:, i * 128:(i + 1) * 128],
                                    vt3[:, h, :],
                                    start=True, stop=True)
                            if BCAST_NORM:
                                rc4 = rcp.tile([128, 4], F32, name="rc4")
                                nc.vector.reciprocal(rc4[:], a3[:, :, HD])
                                ao3 = ao3f[:, HGS[hg][0]:HGS[hg][1]:HGS[hg][2], :]
                                nc.vector.tensor_mul(
                                    ao3, a3[:, :, 0:HD],
                                    rc4.rearrange(
                                        "p (i j) -> p i j", j=1).broadcast_to(
                                        [128, 4, HD]))
                            else:
                                for i, h in enumerate(heads):
                                    rc = rcp.tile([128, 1], F32, name="rc4")
                                    nc.vector.reciprocal(
                                        rc[:], a3[:, i, HD:HD + 1])
                                    nc.vector.tensor_scalar_mul(
                                        ao[:, h * 64:(h + 1) * 64],
                                        a3[:, i, 0:HD], rc[:])
                        else:
                            for i, h in enumerate(heads):
                                aop = aops.tile([128, 128], F32, name="aop4")
                                nc.tensor.matmul(
                                    aop[:, :HD + 1],
                                    att[:, i * 128:(i + 1) * 128],
                                    vt3[:, h, :],
                                    start=True, stop=True)
                                rc = rcp.tile([128, 1], F32, name="rc4")
                                nc.vector.reciprocal(rc[:], aop[:, HD:HD + 1])
                                nc.vector.tensor_scalar_mul(
                                    ao[:, h * 64:(h + 1) * 64],
                                    aop[:, :HD], rc[:])

                    # stage 4: transpose attn_out, proj matmul, store
                    at = atp.tile([128, D], F16, name="at")
                    if DMA_TRANSP:
                        for c in range(8):
                            nc.sync.dma_start_transpose(
                                at[:, c * 128:(c + 1) * 128],
                                ao[:, c * 128:(c + 1) * 128])
                    else:
                        for c in range(8):
                            tpp = tp.tile([128, 128], F16, name="tpp")
                            nc.tensor.transpose(
                                tpp[:], ao[:, c * 128:(c + 1) * 128], ident[:])
                            nc.vector.tensor_copy(
                                at[:, c * 128:(c + 1) * 128], tpp[:])
                    ot = outp.tile([128, D], F32, name="ot")
                    for n in range(2):
                        ac = acc.tile([128, STILE], F32, name="ac")
                        for k in range(8):
                            nc.tensor.matmul(
                                ac[:],
                                at[:, k * 128:(k + 1) * 128],
                                pwts[k][:, n * 512:(n + 1) * 512],
                                start=(k == 0), stop=(k == 7))
                        nc.scalar.copy(ot[:, n * 512:(n + 1) * 512], ac[:])
                    if with_projbias:
                        nc.vector.tensor_add(ot[:], ot[:], pbb[:])
                    nc.sync.dma_start(
                        out=out_ext[s0 + p * 128:s0 + (p + 1) * 128, :],
                        in_=ot[:])

    nc.compile()
    return nc


def _host_prep(x, qkv_w, qkv_b, proj_w, proj_b, rel_bias):
    """Fold scale/biases, cast to fp16, build the exp'd blocked bias table."""
    scale = 1.0 / np.sqrt(HD)
    qkv_w_s = np.asarray(qkv_w, dtype=np.float64).copy()
    qkv_w_s[:, :D] *= scale
    qkv_b = np.asarray(qkv_b, dtype=np.float64)
    qkv_b_s = qkv_b.copy()
    qkv_b_s[:D] *= scale

    # rel-bias expanded to [H, W, W], packed into the transposed,
    # window-pair [128 (k), H*128 (h-major, q)] table, then EXP'd:
    # att = exp(scores) * exp(bias); masked cross-window entries
    # become exactly 0.
    rb = np.asarray(rel_bias, dtype=np.float32)
    coords = np.arange(W)
    rel = coords[:, None] - coords[None, :] + (W - 1)      # [q, k]
    bias_hqk = rb[rel].transpose(2, 0, 1)                  # [H, q, k]
    b2 = np.full((H, 128, 128), MASK, dtype=np.float32)    # [H, k2, q2]
    bias_kq = bias_hqk.transpose(0, 2, 1)                  # [H, k, q]
    b2[:, :64, :64] = bias_kq
    b2[:, 64:, 64:] = bias_kq
    expb16 = np.ascontiguousarray(
        np.exp(b2.transpose(1, 0, 2)).reshape(128, H * 128)).astype(np.float16)

    # v-bias commutes through attention (rows sum to 1) -> fold into proj_b
    proj_b_eff = (qkv_b[2 * D:] @ np.asarray(proj_w, dtype=np.float64)
                  + np.asarray(proj_b, dtype=np.float64))

    shared = {
        "qkvw16": qkv_w_s.astype(np.float16),
        "projw16": np.asarray(proj_w).astype(np.float16),
        "expb16": expb16,
    }
    qk_bias = qkv_b_s[:2 * D]
    with_qkbias = bool(np.any(qk_bias))
    if with_qkbias:
        shared["qkb"] = np.ascontiguousarray(
            qk_bias.reshape(16, 128, 1).astype(np.float32))
    with_projbias = bool(np.any(proj_b_eff))
    if with_projbias:
        shared["projb_bcast"] = np.broadcast_to(
            proj_b_eff.astype(np.float32), (128, D)).copy()
    return shared, with_qkbias, with_projbias


_NC_CACHE = {}


def kernel(x, qkv_w, qkv_b, proj_w, proj_b, rel_bias):
    x = np.asarray(x)
    shared, wqk, wpb = _host_prep(x, qkv_w, qkv_b, proj_w, proj_b, rel_bias)

    key = (wqk, wpb)
    if key not in _NC_CACHE:
        _NC_CACHE[key] = _build(NST, wqk, wpb)
    nc = _NC_CACHE[key]

    # feature-major xT per batch element (seq stays the fast axis on chip)
    xt16 = np.ascontiguousarray(
        x.astype(np.float16).transpose(0, 2, 1))          # [B, D, S]
    in_maps = [dict(shared, xt16=xt16[i]) for i in range(B)]
    res = run_bass_kernel_spmd(nc, in_maps, list(range(B)))
    return np.stack([res.results[i]["out"] for i in range(B)], axis=0)


if __name__ == "__main__":
    rng = np.random.default_rng(0)
    x = rng.standard_normal((B, S, D), dtype=np.float32)
    qkv_w = rng.standard_normal((D, 3 * D), dtype=np.float32) / np.sqrt(D)
    proj_w = rng.standard_normal((D, D), dtype=np.float32) / np.sqrt(D)
    out = kernel(x, qkv_w, np.zeros(3 * D, np.float32), proj_w,
                 np.zeros(D, np.float32),
                 rng.standard_normal((2 * W - 1, H), dtype=np.float32) * 0.02)
    print(out.shape, out.dtype)


# revision 23
# speedup vs baseline: 1.7617x; 1.0036x over previous
"""LocalWindowAttention Trainium2 kernel.

Problem: B=8, S=4096, D=1024, H=16 heads, hd=64, window W=64.
  qkv = x @ qkv_w + qkv_b; per-window attention with relative position
  bias; out = attn_out @ proj_w + proj_b.

Sharding: data-parallel over batch — one batch element per NeuronCore
(8 cores), no collectives needed.

Per-core pipeline (S=4096 rows, processed in s-tiles of 512 rows):
  1. x is pre-transposed HOST-side to xT [D, S]; feature-major xt tiles
     stream in with plain wide DMAs (no on-chip transposes).
  2. qT/kT (feature-major) and v (seq-major) via fp16 matmuls vs resident
     qkv_w tiles; fp32 PSUM accumulation over the K=1024 contraction.
  3. Attention per 128-row block (= 2 windows of 64) and per group of 4
     heads:
       scoresT[k,q] = kT.T @ qT      (4 heads -> 4 col-quarters of ONE
                                      [128,512] PSUM tile; the single
                                      reader comes after all 4 writers,
                                      so no PE-write/DVE-read bank race)
       att = exp(scoresT) * expb     (expb = exp(rel_bias) table, fp16;
                                      cross-window entries are exactly 0,
                                      replacing the -1e4 additive mask)
       outT_unnorm[q,hd], denom[q] = att.T @ [v | 1]   (4 heads -> 4
                                      65-col slots of ONE PSUM tile)
       attn_out[q,hd] = outT_unnorm * (1/denom)  (one strided reciprocal
                                      + one broadcast multiply per group)
  4. attn_out PE-transposed per 128-col block; proj matmul; DMA out.

Scale 1/sqrt(hd) is folded into qkv_w's q-columns host-side. qkv_b's
v-part is folded into an effective proj bias host-side (rows of attn sum
to 1). All matmul operands are fp16 (error ~1e-3 vs fp32 reference);
accumulation is always fp32.
"""
import os
import numpy as np

import concourse.bacc as bacc
import concourse.mybir as mybir
from concourse.tile import TileContext
from concourse.bass_utils import run_bass_kernel_spmd
from concourse.masks import make_identity

F16 = mybir.dt.float16
F32 = mybir.dt.float32

B, S, D = 8, 4096, 1024
H, W, HD = 16, 64, 64
NW = S // W              # 64 windows
STILE = 512              # seq rows per pipeline tile
NST = S // STILE         # 8 s-tiles
NBLK = STILE // 128      # 4 row-blocks (window pairs) per s-tile
MASK = -10000.0          # exp() underflows to exactly 0


# Feature switches (env-overridable for testing). Sharing one PSUM tile
# between 4 score-matmul groups crashed the hardware when the groups MIXED
# stationary base partitions (even heads read kt rows 0-63, odd heads rows
# 64-127); grouping heads by row parity keeps each tile's writers uniform.
SC_SHARED = os.environ.get("KSC", "1") == "1"   # 4 score mms -> one PSUM tile
AV_SHARED = os.environ.get("KAV", "1") == "1"   # 4 av mms -> one PSUM tile
BCAST_NORM = os.environ.get("KBC", "1") == "1"  # batched bcast normalize
DMA_TRANSP = os.environ.get("KDT", "0") == "1"  # attn_out transpose via DMA
                                                # (measured: WORSE — xbar-mode
                                                # switches serialize the DMA
                                                # stream, 732us -> 1057us)
FEAT_MAJOR = os.environ.get("KFM", "0") == "1"  # feature-major av: out[f,q],
                                                # denom in row 64, no attn_out
                                                # transpose stage (measured:
                                                # WORSE — the [1,512] one-
                                                # partition DVE reciprocal on
                                                # the critical path costs
                                                # 3.3us per head-group)

# head groups of 4, uniform kt/qt row parity within each group
HGS = [(0, 8, 2), (1, 8, 2), (8, 16, 2), (9, 16, 2)]


def _build(n_stiles=NST, with_qkbias=False, with_projbias=False):
    nc = bacc.Bacc()
    s_total = n_stiles * STILE

    xt_ext = nc.declare_dram_parameter("xt16", [D, s_total], F16, isOutput=False)
    w_ext = nc.declare_dram_parameter("qkvw16", [D, 3 * D], F16, isOutput=False)
    pw_ext = nc.declare_dram_parameter("projw16", [D, D], F16, isOutput=False)
    eb_ext = nc.declare_dram_parameter("expb16", [128, H * 128], F16,
                                       isOutput=False)
    out_ext = nc.declare_dram_parameter("out", [s_total, D], F32, isOutput=True)
    if with_qkbias:
        qkb_ext = nc.declare_dram_parameter("qkb", [16, 128, 1], F32,
                                            isOutput=False)
    if with_projbias:
        pbb_ext = nc.declare_dram_parameter("projb_bcast", [128, D], F32,
                                            isOutput=False)

    with TileContext(nc) as tc:
        with (
            tc.tile_pool(name="const", bufs=1) as const,
            tc.tile_pool(name="xtp", bufs=2) as xtp,
            tc.tile_pool(name="qktp", bufs=2) as qktp,
            tc.tile_pool(name="vap", bufs=8) as vap,
            tc.tile_pool(name="arp", bufs=6) as arp,
            tc.tile_pool(name="attp", bufs=6) as attp,
            tc.tile_pool(name="rcp", bufs=6) as rcp,
            tc.tile_pool(name="aout", bufs=2) as aout,
            tc.tile_pool(name="atp", bufs=3) as atp,
            tc.tile_pool(name="outp", bufs=2) as outp,
            tc.tile_pool(name="acc", bufs=2, space="PSUM") as acc,
            tc.tile_pool(name="scps", bufs=3, space="PSUM") as scps,
            tc.tile_pool(name="aops", bufs=2, space="PSUM") as aops,
            tc.tile_pool(name="bcps", bufs=2, space="PSUM") as bcps,
            tc.tile_pool(name="tp", bufs=1, space="PSUM") as tp,
        ):
            # ---- resident constants -------------------------------------
            wts = []
            for k in range(8):
                wk = const.tile([128, 3 * D], F16, name=f"wk{k}")
                nc.sync.dma_start(out=wk[:], in_=w_ext[k * 128:(k + 1) * 128, :])
                wts.append(wk)
            ident = const.tile([128, 128], F16, name="ident")
            make_identity(nc, ident)
            onesr = const.tile([1, 64], F16, name="onesr")
            nc.vector.memset(onesr[:], 1.0)
            if with_qkbias:
                qkb = const.tile([128, 16], F32, name="qkb")
                for m in range(16):
                    nc.sync.dma_start(out=qkb[:, m:m + 1], in_=qkb_ext[m])

            # proj weights / bias-exp table aren't needed until the first
            # attention block; emitting their DMA dispatches after stile 0's
            # qkv keeps the critical xt/wts dispatches at the queue head
            pwts, ebt, pbb = [], None, None

            def _late_consts():
                nonlocal ebt, pbb
                for k in range(8):
                    pk = const.tile([128, D], F16, name=f"pk{k}")
                    nc.sync.dma_start(out=pk[:],
                                      in_=pw_ext[k * 128:(k + 1) * 128, :])
                    pwts.append(pk)
                ebt = const.tile([128, H * 128], F16, name="ebt")
                nc.sync.dma_start(out=ebt[:], in_=eb_ext[:])
                if with_projbias:
                    pbb = const.tile([128, D], F32, name="pbb")
                    nc.sync.dma_start(out=pbb[:], in_=pbb_ext[:])

            # ---- main loop over s-tiles ---------------------------------
            for st in range(n_stiles):
                s0 = st * STILE

                # stage 1: feature-major xt, straight from the
                # pre-transposed DRAM copy
                xt = xtp.tile([128, 8 * STILE], F16, name="xt")
                for c in range(8):
                    nc.sync.dma_start(
                        out=xt[:, c * STILE:(c + 1) * STILE],
                        in_=xt_ext[c * 128:(c + 1) * 128, s0:s0 + STILE])

                # stage 2a: qT, kT (feature-major, fp16)
                qt = qktp.tile([128, 8 * STILE], F16, name="qt", tag="qt")
                kt = qktp.tile([128, 8 * STILE], F16, name="kt", tag="kt")
                # m-blocks interleaved q,k so attention head-groups (which
                # need q AND k of m 0-3 first) unblock after 8 drains, not 12
                for m in range(8):
                    for which, dst in ((0, qt), (1, kt)):
                        ac = acc.tile([128, STILE], F32, name="ac")
                        col0 = which * D + m * 128
                        for k in range(8):
                            nc.tensor.matmul(
                                ac[:],
                                wts[k][:, col0:col0 + 128],
                                xt[:, k * STILE:(k + 1) * STILE],
                                start=(k == 0), stop=(k == 7))
                        dsl = dst[:, m * STILE:(m + 1) * STILE]
                        if with_qkbias:
                            nc.scalar.activation(
                                dsl, ac[:], mybir.ActivationFunctionType.Identity,
                                bias=qkb[:, which * 8 + m:which * 8 + m + 1])
                        elif m % 2 == which:
                            nc.scalar.copy(dsl, ac[:])
                        else:
                            nc.vector.tensor_copy(dsl, ac[:])

                if st == 0:
                    _late_consts()

                # stage 2b: v (seq-major, ones column appended per head)
                vts = []
                for b in range(NBLK):
                    vt = vap.tile([128, H * (HD + 1)], F16, name="vt")
                    vt3 = vt.rearrange("p (h c) -> p h c", c=HD + 1)
                    nc.vector.memset(vt3[:, :, HD:HD + 1], 1.0)
                    for n in range(2):
                        ac = acc.tile([128, STILE], F32, name="ac")
                        for k in range(8):
                            nc.tensor.matmul(
                                ac[:],
                                xt[:, k * STILE + b * 128:k * STILE + b * 128 + 128],
                                wts[k][:, 2 * D + n * 512:2 * D + (n + 1) * 512],
                                start=(k == 0), stop=(k == 7))
                        nc.vector.tensor_copy(
                            vt3[:, n * 8:(n + 1) * 8, 0:HD],
                            ac.rearrange("p (h c) -> p h c", c=HD))
                    vts.append(vt)

                # stage 3: attention per window-pair block, 4 heads at a time
                for p in range(NBLK):
                    ebt3 = ebt.rearrange("p (h q) -> p h q", q=128)
                    vt3 = vts[p].rearrange("p (h c) -> p h c", c=HD + 1)
                    if FEAT_MAJOR:
                        # at[f-in-block, 8 blocks x 128 q] is written
                        # directly by the normalization multiplies; each
                        # parity head-group covers one [64, 512] quadrant
                        at = atp.tile([128, D], F16, name="at")
                        for hg in range(4):
                            heads = list(range(*HGS[hg]))
                            r = (heads[0] % 2) * 64
                            araw = arp.tile([128, 512], F16, name="araw")
                            scb = scps.tile([128, 512], F32, name="scb")
                            for i, h in enumerate(heads):
                                c0 = (h // 2) * STILE + p * 128
                                nc.tensor.matmul(
                                    scb[:, i * 128:(i + 1) * 128],
                                    kt[r:r + 64, c0:c0 + 128],
                                    qt[r:r + 64, c0:c0 + 128],
                                    start=True, stop=True)
                            nc.scalar.activation(
                                araw[:], scb[:],
                                mybir.ActivationFunctionType.Exp)
                            att = attp.tile([128, 512], F16, name="att")
                            nc.vector.tensor_mul(
                                att[:], araw[:],
                                ebt3[:, HGS[hg][0]:HGS[hg][1]:HGS[hg][2], :])
                            # v-stationary av: out [65, 128q] per head; all
                            # four share one PSUM bank at base partition 0;
                            # row 64 = softmax denominators
                            av4 = aops.tile([65, 512], F32, name="aop4")
                            for i, h in enumerate(heads):
                                nc.tensor.matmul(
                                    av4[:, i * 128:(i + 1) * 128],
                                    vt3[:, h, :],
                                    att[:, i * 128:(i + 1) * 128],
                                    start=True, stop=True)
                            rcr = rcp.tile([1, 512], F16, name="rcr")
                            with nc.allow_low_precision(
                                    reason="softmax reciprocal fp16"):
                                nc.vector.reciprocal(rcr[:], av4[64:65, :])
                            # DVE can't read stride-0 partition APs, so
                            # materialize 1 (x) rc with a K=1 outer-product
                            # matmul; DVE also can't read two PSUM inputs,
                            # so the unnormalized rows drain to SBUF via the
                            # scalar engine (parallel to the rc chain)
                            rcb = bcps.tile([64, 512], F32, name="rcb")
                            nc.tensor.matmul(rcb[:], onesr[:], rcr[:],
                                             start=True, stop=True)
                            avs = arp.tile([64, 512], F16, name="avs",
                                           tag="avs")
                            nc.scalar.copy(avs[:], av4[0:64, :])
                            cb = 0 if hg < 2 else 512
                            nc.vector.tensor_mul(
                                at[r:r + 64, cb:cb + 512],
                                avs[:], rcb[:])
                        ot = outp.tile([128, D], F32, name="ot")
                        for n in range(2):
                            ac = acc.tile([128, STILE], F32, name="ac")
                            for k in range(8):
                                nc.tensor.matmul(
                                    ac[:],
                                    at[:, k * 128:(k + 1) * 128],
                                    pwts[k][:, n * 512:(n + 1) * 512],
                                    start=(k == 0), stop=(k == 7))
                            nc.scalar.copy(ot[:, n * 512:(n + 1) * 512], ac[:])
                        if with_projbias:
                            nc.vector.tensor_add(ot[:], ot[:], pbb[:])
                        nc.sync.dma_start(
                            out=out_ext[s0 + p * 128:s0 + (p + 1) * 128, :],
                            in_=ot[:])
                        continue
                    ao = aout.tile([128, D], F16, name="ao")
                    ao3f = ao.rearrange("p (h c) -> p h c", c=HD)
                    for hg in range(4):
                        heads = list(range(*HGS[hg]))
                        r = (heads[0] % 2) * 64
                        araw = arp.tile([128, 512], F16, name="araw")
                        if SC_SHARED:
                            # 4 score matmuls -> col-quarters of one PSUM
                            # tile (uniform stationary base partition); the
                            # only reader (exp) runs after all four
                            scb = scps.tile([128, 512], F32, name="scb")
                            for i, h in enumerate(heads):
                                c0 = (h // 2) * STILE + p * 128
                                nc.tensor.matmul(
                                    scb[:, i * 128:(i + 1) * 128],
                                    kt[r:r + 64, c0:c0 + 128],
                                    qt[r:r + 64, c0:c0 + 128],
                                    start=True, stop=True)
                            for half in range(2):
                                hs = slice(half * 256, (half + 1) * 256)
                                nc.scalar.activation(
                                    araw[:, hs], scb[:, hs],
                                    mybir.ActivationFunctionType.Exp)
                        else:
                            for i, h in enumerate(heads):
                                c0 = (h // 2) * STILE + p * 128
                                sc = scps.tile([128, 128], F32, name="scb")
                                nc.tensor.matmul(
                                    sc[:],
                                    kt[r:r + 64, c0:c0 + 128],
                                    qt[r:r + 64, c0:c0 + 128],
                                    start=True, stop=True)
                                nc.scalar.activation(
                                    araw[:, i * 128:(i + 1) * 128], sc[:],
                                    mybir.ActivationFunctionType.Exp)
                        att = attp.tile([128, 512], F16, name="att")
                        eb_hg = ebt3[:, HGS[hg][0]:HGS[hg][1]:HGS[hg][2], :]
                        for half in range(2):
                            hs = slice(half * 256, (half + 1) * 256)
                            nc.vector.tensor_mul(
                                att[:, hs].rearrange("p (h q) -> p h q", q=128),
                                araw[:, hs].rearrange("p (h q) -> p h q", q=128),
                                eb_hg[:, half * 2:(half + 1) * 2, :])
                        if AV_SHARED:
                            # 4 attention-weighted-V matmuls -> 65-col slots
                            # of one PSUM tile (col 64 of each = denominator)
                            aop4 = aops.tile([128, 4 * (HD + 1)], F32,
                                             name="aop4")
                            a3 = aop4.rearrange("p (i c) -> p i c", c=HD + 1)
                            for i, h in enumerate(heads):
                                nc.tensor.matmul(
                                    aop4[:, i * (HD + 1):(i + 1) * (HD + 1)],
                                    att[:, i * 128:(i + 1) * 128],
                                    vt3[:, h, :],
                                    start=True, stop=True)
                            if BCAST_NORM:
                                rc4 = rcp.tile([128, 4], F32, name="rc4")
                                nc.vector.reciprocal(rc4[:], a3[:, :, HD])
                                ao3 = ao3f[:, HGS[hg][0]:HGS[hg][1]:HGS[hg][2], :]
                                nc.vector.tensor_mul(
                                    ao3, a3[:, :, 0:HD],
                                    rc4.rearrange(
                                        "p (i j) -> p i j", j=1).broadcast_to(
                                        [128, 4, HD]))
                            else:
                                for i, h in enumerate(heads):
                                    rc = rcp.tile([128, 1], F32, name="rc4")
                                    nc.vector.reciprocal(
                                        rc[:], a3[:, i, HD:HD + 1])
                                    nc.vector.tensor_scalar_mul(
                                        ao[:, h * 64:(h + 1) * 64],
                                        a3[:, i, 0:HD], rc[:])
                        else:
                            for i, h in enumerate(heads):
                                aop = aops.tile([128, 128], F32, name="aop4")
                                nc.tensor.matmul(
                                    aop[:, :HD + 1],
                                    att[:, i * 128:(i + 1) * 128],
                                    vt3[:, h, :],
                                    start=True, stop=True)
                                rc = rcp.tile([128, 1], F32, name="rc4")
                                nc.vector.reciprocal(rc[:], aop[:, HD:HD + 1])
                                nc.vector.tensor_scalar_mul(
                                    ao[:, h * 64:(h + 1) * 64],
                                    aop[:, :HD], rc[:])

                    # stage 4: transpose attn_out, proj matmul, store
                    at = atp.tile([128, D], F16, name="at")
                    if DMA_TRANSP:
                        for c in range(8):
                            nc.sync.dma_start_transpose(
                                at[:, c * 128:(c + 1) * 128],
                                ao[:, c * 128:(c + 1) * 128])
                    else:
                        for c in range(8):
                            tpp = tp.tile([128, 128], F16, name="tpp")
                            nc.tensor.transpose(
                                tpp[:], ao[:, c * 128:(c + 1) * 128], ident[:])
                            nc.vector.tensor_copy(
                                at[:, c * 128:(c + 1) * 128], tpp[:])
                    ot = outp.tile([128, D], F32, name="ot")
                    for n in range(2):
                        ac = acc.tile([128, STILE], F32, name="ac")
                        for k in range(8):
                            nc.tensor.matmul(
                                ac[:],
                                at[:, k * 128:(k + 1) * 128],
                                pwts[k][:, n * 512:(n + 1) * 512],
                                start=(k == 0), stop=(k == 7))
                        nc.scalar.copy(ot[:, n * 512:(n + 1) * 512], ac[:])
                    if with_projbias:
                        nc.vector.tensor_add(ot[:], ot[:], pbb[:])
                    nc.sync.dma_start(
                        out=out_ext[s0 + p * 128:s0 + (p + 1) * 128, :],
                        in_=ot[:])

    nc.compile()
    return nc


def _host_prep(x, qkv_w, qkv_b, proj_w, proj_b, rel_bias):
    """Fold scale/biases, cast to fp16, build the exp'd blocked bias table."""
    scale = 1.0 / np.sqrt(HD)
    qkv_w_s = np.asarray(qkv_w, dtype=np.float64).copy()
    qkv_w_s[:, :D] *= scale
    qkv_b = np.asarray(qkv_b, dtype=np.float64)
    qkv_b_s = qkv_b.copy()
    qkv_b_s[:D] *= scale

    # rel-bias expanded to [H, W, W], packed into the transposed,
    # window-pair [128 (k), H*128 (h-major, q)] table, then EXP'd:
    # att = exp(scores) * exp(bias); masked cross-window entries
    # become exactly 0.
    rb = np.asarray(rel_bias, dtype=np.float32)
    coords = np.arange(W)
    rel = coords[:, None] - coords[None, :] + (W - 1)      # [q, k]
    bias_hqk = rb[rel].transpose(2, 0, 1)                  # [H, q, k]
    b2 = np.full((H, 128, 128), MASK, dtype=np.float32)    # [H, k2, q2]
    bias_kq = bias_hqk.transpose(0, 2, 1)                  # [H, k, q]
    b2[:, :64, :64] = bias_kq
    b2[:, 64:, 64:] = bias_kq
    expb16 = np.ascontiguousarray(
        np.exp(b2.transpose(1, 0, 2)).reshape(128, H * 128)).astype(np.float16)

    # v-bias commutes through attention (rows sum to 1) -> fold into proj_b
    proj_b_eff = (qkv_b[2 * D:] @ np.asarray(proj_w, dtype=np.float64)
                  + np.asarray(proj_b, dtype=np.float64))

    shared = {
        "qkvw16": qkv_w_s.astype(np.float16),
        "projw16": np.asarray(proj_w).astype(np.float16),
        "expb16": expb16,
    }
    qk_bias = qkv_b_s[:2 * D]
    with_qkbias = bool(np.any(qk_bias))
    if with_qkbias:
        shared["qkb"] = np.ascontiguousarray(
            qk_bias.reshape(16, 128, 1).astype(np.float32))
    with_projbias = bool(np.any(proj_b_eff))
    if with_projbias:
        shared["projb_bcast"] = np.broadcast_to(
            proj_b_eff.astype(np.float32), (128, D)).copy()
    return shared, with_qkbias, with_projbias


_NC_CACHE = {}


def kernel(x, qkv_w, qkv_b, proj_w, proj_b, rel_bias):
    x = np.asarray(x)
    shared, wqk, wpb = _host_prep(x, qkv_w, qkv_b, proj_w, proj_b, rel_bias)

    key = (wqk, wpb)
    if key not in _NC_CACHE:
        _NC_CACHE[key] = _build(NST, wqk, wpb)
    nc = _NC_CACHE[key]

    # feature-major xT per batch element (seq stays the fast axis on chip)
    xt16 = np.ascontiguousarray(
        x.astype(np.float16).transpose(0, 2, 1))          # [B, D, S]
    in_maps = [dict(shared, xt16=xt16[i]) for i in range(B)]
    res = run_bass_kernel_spmd(nc, in_maps, list(range(B)))
    return np.stack([res.results[i]["out"] for i in range(B)], axis=0)


if __name__ == "__main__":
    rng = np.random.default_rng(0)
    x = rng.standard_normal((B, S, D), dtype=np.float32)
    qkv_w = rng.standard_normal((D, 3 * D), dtype=np.float32) / np.sqrt(D)
    proj_w = rng.standard_normal((D, D), dtype=np.float32) / np.sqrt(D)
    out = kernel(x, qkv_w, np.zeros(3 * D, np.float32), proj_w,
                 np.zeros(D, np.float32),
                 rng.standard_normal((2 * W - 1, H), dtype=np.float32) * 0.02)
    print(out.shape, out.dtype)


# revision 25
# speedup vs baseline: 1.8445x; 1.0470x over previous
"""LocalWindowAttention Trainium2 kernel.

Problem: B=8, S=4096, D=1024, H=16 heads, hd=64, window W=64.
  qkv = x @ qkv_w + qkv_b; per-window attention with relative position
  bias; out = attn_out @ proj_w + proj_b.

Sharding: data-parallel over batch — one batch element per NeuronCore
(8 cores), no collectives needed.

Per-core pipeline (S=4096 rows, processed in s-tiles of 512 rows):
  1. x is pre-transposed HOST-side to xT [D, S]; feature-major xt tiles
     stream in with plain wide DMAs (no on-chip transposes).
  2. qT/kT (feature-major) and v (seq-major) via fp16 matmuls vs resident
     qkv_w tiles; fp32 PSUM accumulation over the K=1024 contraction.
  3. Attention per 128-row block (= 2 windows of 64) and per group of 4
     heads:
       scoresT[k,q] = kT.T @ qT      (4 heads -> 4 col-quarters of ONE
                                      [128,512] PSUM tile; the single
                                      reader comes after all 4 writers,
                                      so no PE-write/DVE-read bank race)
       att = exp(scoresT) * expb     (expb = exp(rel_bias) table, fp16;
                                      cross-window entries are exactly 0,
                                      replacing the -1e4 additive mask)
       outT_unnorm[q,hd], denom[q] = att.T @ [v | 1]   (4 heads -> 4
                                      65-col slots of ONE PSUM tile)
       attn_out[q,hd] = outT_unnorm * (1/denom)  (one strided reciprocal
                                      + one broadcast multiply per group)
  4. attn_out PE-transposed per 128-col block; proj matmul; DMA out.

Scale 1/sqrt(hd) is folded into qkv_w's q-columns host-side. qkv_b's
v-part is folded into an effective proj bias host-side (rows of attn sum
to 1). All matmul operands are fp16 (error ~1e-3 vs fp32 reference);
accumulation is always fp32.
"""
import os
import numpy as np

import concourse.bacc as bacc
import concourse.mybir as mybir
from concourse.tile import TileContext
from concourse.bass_utils import run_bass_kernel_spmd
from concourse.masks import make_identity

F16 = mybir.dt.float16
F32 = mybir.dt.float32

B, S, D = 8, 4096, 1024
H, W, HD = 16, 64, 64
NW = S // W              # 64 windows
STILE = 512              # seq rows per pipeline tile
NST = S // STILE         # 8 s-tiles
NBLK = STILE // 128      # 4 row-blocks (window pairs) per s-tile
MASK = -10000.0          # exp() underflows to exactly 0


# Feature switches (env-overridable for testing). Sharing one PSUM tile
# between 4 score-matmul groups crashed the hardware when the groups MIXED
# stationary base partitions (even heads read kt rows 0-63, odd heads rows
# 64-127); grouping heads by row parity keeps each tile's writers uniform.
SC_SHARED = os.environ.get("KSC", "1") == "1"   # 4 score mms -> one PSUM tile
AV_SHARED = os.environ.get("KAV", "1") == "1"   # 4 av mms -> one PSUM tile
BCAST_NORM = os.environ.get("KBC", "1") == "1"  # batched bcast normalize
DMA_TRANSP = os.environ.get("KDT", "0") == "1"  # attn_out transpose via DMA
                                                # (measured: WORSE — xbar-mode
                                                # switches serialize the DMA
                                                # stream, 732us -> 1057us)
FEAT_MAJOR = os.environ.get("KFM", "0") == "1"  # feature-major av: out[f,q],
                                                # denom in row 64, no attn_out
                                                # transpose stage (measured:
                                                # WORSE — the [1,512] one-
                                                # partition DVE reciprocal on
                                                # the critical path costs
                                                # 3.3us per head-group)

# head groups of 4, uniform kt/qt row parity within each group
HGS = [(0, 8, 2), (1, 8, 2), (8, 16, 2), (9, 16, 2)]


def _build(n_stiles=NST, with_qkbias=False, with_projbias=False):
    nc = bacc.Bacc()
    s_total = n_stiles * STILE

    xt_ext = nc.declare_dram_parameter("xt16", [D, s_total], F16, isOutput=False)
    w_ext = nc.declare_dram_parameter("qkvw16", [D, 3 * D], F16, isOutput=False)
    pw_ext = nc.declare_dram_parameter("projw16", [D, D], F16, isOutput=False)
    eb_ext = nc.declare_dram_parameter("expb16", [128, H * 128], F16,
                                       isOutput=False)
    out_ext = nc.declare_dram_parameter("out", [s_total, D], F32, isOutput=True)
    if with_qkbias:
        qkb_ext = nc.declare_dram_parameter("qkb", [16, 128, 1], F32,
                                            isOutput=False)
    if with_projbias:
        pbb_ext = nc.declare_dram_parameter("projb_bcast", [128, D], F32,
                                            isOutput=False)

    with TileContext(nc) as tc:
        with (
            tc.tile_pool(name="const", bufs=1) as const,
            tc.tile_pool(name="xtp", bufs=2) as xtp,
            tc.tile_pool(name="qktp", bufs=2) as qktp,
            tc.tile_pool(name="vap", bufs=8) as vap,
            tc.tile_pool(name="arp", bufs=6) as arp,
            tc.tile_pool(name="attp", bufs=6) as attp,
            tc.tile_pool(name="rcp", bufs=6) as rcp,
            tc.tile_pool(name="aout", bufs=2) as aout,
            tc.tile_pool(name="atp", bufs=3) as atp,
            tc.tile_pool(name="outp", bufs=2) as outp,
            tc.tile_pool(name="acc", bufs=2, space="PSUM") as acc,
            tc.tile_pool(name="scps", bufs=3, space="PSUM") as scps,
            tc.tile_pool(name="aops", bufs=2, space="PSUM") as aops,
            tc.tile_pool(name="bcps", bufs=2, space="PSUM") as bcps,
            tc.tile_pool(name="tp", bufs=1, space="PSUM") as tp,
        ):
            # ---- resident constants -------------------------------------
            wts = []
            for k in range(8):
                wk = const.tile([128, 3 * D], F16, name=f"wk{k}")
                nc.sync.dma_start(out=wk[:], in_=w_ext[k * 128:(k + 1) * 128, :])
                wts.append(wk)
            ident = const.tile([128, 128], F16, name="ident")
            make_identity(nc, ident)
            onesr = const.tile([1, 64], F16, name="onesr")
            nc.vector.memset(onesr[:], 1.0)
            if with_qkbias:
                qkb = const.tile([128, 16], F32, name="qkb")
                for m in range(16):
                    nc.sync.dma_start(out=qkb[:, m:m + 1], in_=qkb_ext[m])

            # proj weights / bias-exp table aren't needed until the first
            # attention block; emitting their DMA dispatches after stile 0's
            # qkv keeps the critical xt/wts dispatches at the queue head
            pwts, ebt, pbb = [], None, None

            def _late_consts():
                nonlocal ebt, pbb
                for k in range(8):
                    pk = const.tile([128, D], F16, name=f"pk{k}")
                    nc.sync.dma_start(out=pk[:],
                                      in_=pw_ext[k * 128:(k + 1) * 128, :])
                    pwts.append(pk)
                ebt = const.tile([128, H * 128], F16, name="ebt")
                nc.sync.dma_start(out=ebt[:], in_=eb_ext[:])
                if with_projbias:
                    pbb = const.tile([128, D], F32, name="pbb")
                    nc.sync.dma_start(out=pbb[:], in_=pbb_ext[:])

            def _attention(vts, qt, kt, s0):

                for p in range(NBLK):
                    ebt3 = ebt.rearrange("p (h q) -> p h q", q=128)
                    vt3 = vts[p].rearrange("p (h c) -> p h c", c=HD + 1)
                    if FEAT_MAJOR:
                        # at[f-in-block, 8 blocks x 128 q] is written
                        # directly by the normalization multiplies; each
                        # parity head-group covers one [64, 512] quadrant
                        at = atp.tile([128, D], F16, name="at")
                        for hg in range(4):
                            heads = list(range(*HGS[hg]))
                            r = (heads[0] % 2) * 64
                            araw = arp.tile([128, 512], F16, name="araw")
                            scb = scps.tile([128, 512], F32, name="scb")
                            for i, h in enumerate(heads):
                                c0 = (h // 2) * STILE + p * 128
                                nc.tensor.matmul(
                                    scb[:, i * 128:(i + 1) * 128],
                                    kt[r:r + 64, c0:c0 + 128],
                                    qt[r:r + 64, c0:c0 + 128],
                                    start=True, stop=True)
                            nc.scalar.activation(
                                araw[:], scb[:],
                                mybir.ActivationFunctionType.Exp)
                            att = attp.tile([128, 512], F16, name="att")
                            nc.vector.tensor_mul(
                                att[:], araw[:],
                                ebt3[:, HGS[hg][0]:HGS[hg][1]:HGS[hg][2], :])
                            # v-stationary av: out [65, 128q] per head; all
                            # four share one PSUM bank at base partition 0;
                            # row 64 = softmax denominators
                            av4 = aops.tile([65, 512], F32, name="aop4")
                            for i, h in enumerate(heads):
                                nc.tensor.matmul(
                                    av4[:, i * 128:(i + 1) * 128],
                                    vt3[:, h, :],
                                    att[:, i * 128:(i + 1) * 128],
                                    start=True, stop=True)
                            rcr = rcp.tile([1, 512], F16, name="rcr")
                            with nc.allow_low_precision(
                                    reason="softmax reciprocal fp16"):
                                nc.vector.reciprocal(rcr[:], av4[64:65, :])
                            # DVE can't read stride-0 partition APs, so
                            # materialize 1 (x) rc with a K=1 outer-product
                            # matmul; DVE also can't read two PSUM inputs,
                            # so the unnormalized rows drain to SBUF via the
                            # scalar engine (parallel to the rc chain)
                            rcb = bcps.tile([64, 512], F32, name="rcb")
                            nc.tensor.matmul(rcb[:], onesr[:], rcr[:],
                                             start=True, stop=True)
                            avs = arp.tile([64, 512], F16, name="avs",
                                           tag="avs")
                            nc.scalar.copy(avs[:], av4[0:64, :])
                            cb = 0 if hg < 2 else 512
                            nc.vector.tensor_mul(
                                at[r:r + 64, cb:cb + 512],
                                avs[:], rcb[:])
                        ot = outp.tile([128, D], F32, name="ot")
                        for n in range(2):
                            ac = acc.tile([128, STILE], F32, name="ac")
                            for k in range(8):
                                nc.tensor.matmul(
                                    ac[:],
                                    at[:, k * 128:(k + 1) * 128],
                                    pwts[k][:, n * 512:(n + 1) * 512],
                                    start=(k == 0), stop=(k == 7))
                            nc.scalar.copy(ot[:, n * 512:(n + 1) * 512], ac[:])
                        if with_projbias:
                            nc.vector.tensor_add(ot[:], ot[:], pbb[:])
                        nc.sync.dma_start(
                            out=out_ext[s0 + p * 128:s0 + (p + 1) * 128, :],
                            in_=ot[:])
                        continue
                    ao = aout.tile([128, D], F16, name="ao")
                    ao3f = ao.rearrange("p (h c) -> p h c", c=HD)
                    for hg in range(4):
                        heads = list(range(*HGS[hg]))
                        r = (heads[0] % 2) * 64
                        araw = arp.tile([128, 512], F16, name="araw")
                        if SC_SHARED:
                            # 4 score matmuls -> col-quarters of one PSUM
                            # tile (uniform stationary base partition); the
                            # only reader (exp) runs after all four
                            scb = scps.tile([128, 512], F32, name="scb")
                            for i, h in enumerate(heads):
                                c0 = (h // 2) * STILE + p * 128
                                nc.tensor.matmul(
                                    scb[:, i * 128:(i + 1) * 128],
                                    kt[r:r + 64, c0:c0 + 128],
                                    qt[r:r + 64, c0:c0 + 128],
                                    start=True, stop=True)
                            for half in range(2):
                                hs = slice(half * 256, (half + 1) * 256)
                                nc.scalar.activation(
                                    araw[:, hs], scb[:, hs],
                                    mybir.ActivationFunctionType.Exp)
                        else:
                            for i, h in enumerate(heads):
                                c0 = (h // 2) * STILE + p * 128
                                sc = scps.tile([128, 128], F32, name="scb")
                                nc.tensor.matmul(
                                    sc[:],
                                    kt[r:r + 64, c0:c0 + 128],
                                    qt[r:r + 64, c0:c0 + 128],
                                    start=True, stop=True)
                                nc.scalar.activation(
                                    araw[:, i * 128:(i + 1) * 128], sc[:],
                                    mybir.ActivationFunctionType.Exp)
                        att = attp.tile([128, 512], F16, name="att")
                        eb_hg = ebt3[:, HGS[hg][0]:HGS[hg][1]:HGS[hg][2], :]
                        for half in range(2):
                            hs = slice(half * 256, (half + 1) * 256)
                            nc.vector.tensor_mul(
                                att[:, hs].rearrange("p (h q) -> p h q", q=128),
                                araw[:, hs].rearrange("p (h q) -> p h q", q=128),
                                eb_hg[:, half * 2:(half + 1) * 2, :])
                        if AV_SHARED:
                            # 4 attention-weighted-V matmuls -> 65-col slots
                            # of one PSUM tile (col 64 of each = denominator)
                            aop4 = aops.tile([128, 4 * (HD + 1)], F32,
                                             name="aop4")
                            a3 = aop4.rearrange("p (i c) -> p i c", c=HD + 1)
                            for i, h in enumerate(heads):
                                nc.tensor.matmul(
                                    aop4[:, i * (HD + 1):(i + 1) * (HD + 1)],
                                    att[:, i * 128:(i + 1) * 128],
                                    vt3[:, h, :],
                                    start=True, stop=True)
                            if BCAST_NORM:
                                rc4 = rcp.tile([128, 4], F32, name="rc4")
                                nc.vector.reciprocal(rc4[:], a3[:, :, HD])
                                ao3 = ao3f[:, HGS[hg][0]:HGS[hg][1]:HGS[hg][2], :]
                                nc.vector.tensor_mul(
                                    ao3, a3[:, :, 0:HD],
                                    rc4.rearrange(
                                        "p (i j) -> p i j", j=1).broadcast_to(
                                        [128, 4, HD]))
                            else:
                                for i, h in enumerate(heads):
                                    rc = rcp.tile([128, 1], F32, name="rc4")
                                    nc.vector.reciprocal(
                                        rc[:], a3[:, i, HD:HD + 1])
                                    nc.vector.tensor_scalar_mul(
                                        ao[:, h * 64:(h + 1) * 64],
                                        a3[:, i, 0:HD], rc[:])
                        else:
                            for i, h in enumerate(heads):
                                aop = aops.tile([128, 128], F32, name="aop4")
                                nc.tensor.matmul(
                                    aop[:, :HD + 1],
                                    att[:, i * 128:(i + 1) * 128],
                                    vt3[:, h, :],
                                    start=True, stop=True)
                                rc = rcp.tile([128, 1], F32, name="rc4")
                                nc.vector.reciprocal(rc[:], aop[:, HD:HD + 1])
                                nc.vector.tensor_scalar_mul(
                                    ao[:, h * 64:(h + 1) * 64],
                                    aop[:, :HD], rc[:])

                    # stage 4: transpose attn_out, proj matmul, store
                    at = atp.tile([128, D], F16, name="at")
                    if DMA_TRANSP:
                        for c in range(8):
                            nc.sync.dma_start_transpose(
                                at[:, c * 128:(c + 1) * 128],
                                ao[:, c * 128:(c + 1) * 128])
                    else:
                        for c in range(8):
                            tpp = tp.tile([128, 128], F16, name="tpp")
                            nc.tensor.transpose(
                                tpp[:], ao[:, c * 128:(c + 1) * 128], ident[:])
                            nc.vector.tensor_copy(
                                at[:, c * 128:(c + 1) * 128], tpp[:])
                    ot = outp.tile([128, D], F32, name="ot")
                    for n in range(2):
                        ac = acc.tile([128, STILE], F32, name="ac")
                        for k in range(8):
                            nc.tensor.matmul(
                                ac[:],
                                at[:, k * 128:(k + 1) * 128],
                                pwts[k][:, n * 512:(n + 1) * 512],
                                start=(k == 0), stop=(k == 7))
                        nc.scalar.copy(ot[:, n * 512:(n + 1) * 512], ac[:])
                    if with_projbias:
                        nc.vector.tensor_add(ot[:], ot[:], pbb[:])
                    nc.sync.dma_start(
                        out=out_ext[s0 + p * 128:s0 + (p + 1) * 128, :],
                        in_=ot[:])


            # ---- main loop over s-tiles (+1 epilogue pass) --------------
            prev_stage = None
            for st in range(n_stiles + 1):
                s0 = st * STILE

                # stage 1: feature-major xt, straight from the
                # pre-transposed DRAM copy
                if st == n_stiles:
                    # epilogue pass: no new qkv, just the last attention
                    (vts, qt, kt, s0) = prev_stage
                    _attention(vts, qt, kt, s0)
                    break
                xt = xtp.tile([128, 8 * STILE], F16, name="xt")
                for c in range(8):
                    nc.sync.dma_start(
                        out=xt[:, c * STILE:(c + 1) * STILE],
                        in_=xt_ext[c * 128:(c + 1) * 128, s0:s0 + STILE])

                # stage 2a: qT, kT (feature-major, fp16)
                qt = qktp.tile([128, 8 * STILE], F16, name="qt", tag="qt")
                kt = qktp.tile([128, 8 * STILE], F16, name="kt", tag="kt")
                # m-blocks interleaved q,k so attention head-groups (which
                # need q AND k of m 0-3 first) unblock after 8 drains, not 12
                for m in range(8):
                    for which, dst in ((0, qt), (1, kt)):
                        ac = acc.tile([128, STILE], F32, name="ac")
                        col0 = which * D + m * 128
                        for k in range(8):
                            nc.tensor.matmul(
                                ac[:],
                                wts[k][:, col0:col0 + 128],
                                xt[:, k * STILE:(k + 1) * STILE],
                                start=(k == 0), stop=(k == 7))
                        dsl = dst[:, m * STILE:(m + 1) * STILE]
                        if with_qkbias:
                            nc.scalar.activation(
                                dsl, ac[:], mybir.ActivationFunctionType.Identity,
                                bias=qkb[:, which * 8 + m:which * 8 + m + 1])
                        else:
                            nc.scalar.copy(dsl, ac[:])

                if st == 0:
                    _late_consts()

                # stage 2b: v (seq-major, ones column appended per head)
                vts = []
                for b in range(NBLK):
                    vt = vap.tile([128, H * (HD + 1)], F16, name="vt")
                    vt3 = vt.rearrange("p (h c) -> p h c", c=HD + 1)
                    nc.vector.memset(vt3[:, :, HD:HD + 1], 1.0)
                    for n in range(2):
                        ac = acc.tile([128, STILE], F32, name="ac")
                        for k in range(8):
                            nc.tensor.matmul(
                                ac[:],
                                xt[:, k * STILE + b * 128:k * STILE + b * 128 + 128],
                                wts[k][:, 2 * D + n * 512:2 * D + (n + 1) * 512],
                                start=(k == 0), stop=(k == 7))
                        nc.vector.tensor_copy(
                            vt3[:, n * 8:(n + 1) * 8, 0:HD],
                            ac.rearrange("p (h c) -> p h c", c=HD))
                    vts.append(vt)

                # stage 3+4 for the PREVIOUS s-tile: emitting the current
                # tile's qkv first means its qt/kt drains complete while the
                # previous attention still has PE work, killing the
                # stile-boundary stalls (software-pipeline inversion)
                cur = (vts, qt, kt, s0)
                if prev_stage is None:
                    prev_stage = cur
                    continue
                (vts, qt, kt, s0), prev_stage = prev_stage, cur
                _attention(vts, qt, kt, s0)

    nc.compile()
    return nc


def _host_prep(x, qkv_w, qkv_b, proj_w, proj_b, rel_bias):
    """Fold scale/biases, cast to fp16, build the exp'd blocked bias table."""
    scale = 1.0 / np.sqrt(HD)
    qkv_w_s = np.asarray(qkv_w, dtype=np.float64).copy()
    qkv_w_s[:, :D] *= scale
    qkv_b = np.asarray(qkv_b, dtype=np.float64)
    qkv_b_s = qkv_b.copy()
    qkv_b_s[:D] *= scale

    # rel-bias expanded to [H, W, W], packed into the transposed,
    # window-pair [128 (k), H*128 (h-major, q)] table, then EXP'd:
    # att = exp(scores) * exp(bias); masked cross-window entries
    # become exactly 0.
    rb = np.asarray(rel_bias, dtype=np.float32)
    coords = np.arange(W)
    rel = coords[:, None] - coords[None, :] + (W - 1)      # [q, k]
    bias_hqk = rb[rel].transpose(2, 0, 1)                  # [H, q, k]
    b2 = np.full((H, 128, 128), MASK, dtype=np.float32)    # [H, k2, q2]
    bias_kq = bias_hqk.transpose(0, 2, 1)                  # [H, k, q]
    b2[:, :64, :64] = bias_kq
    b2[:, 64:, 64:] = bias_kq
    expb16 = np.ascontiguousarray(
        np.exp(b2.transpose(1, 0, 2)).reshape(128, H * 128)).astype(np.float16)

    # v-bias commutes through attention (rows sum to 1) -> fold into proj_b
    proj_b_eff = (qkv_b[2 * D:] @ np.asarray(proj_w, dtype=np.float64)
                  + np.asarray(proj_b, dtype=np.float64))

    shared = {
        "qkvw16": qkv_w_s.astype(np.float16),
        "projw16": np.asarray(proj_w).astype(np.float16),
        "expb16": expb16,
    }
    qk_bias = qkv_b_s[:2 * D]
    with_qkbias = bool(np.any(qk_bias))
    if with_qkbias:
        shared["qkb"] = np.ascontiguousarray(
            qk_bias.reshape(16, 128, 1).astype(np.float32))
    with_projbias = bool(np.any(proj_b_eff))
    if with_projbias:
        shared["projb_bcast"] = np.broadcast_to(
            proj_b_eff.astype(np.float32), (128, D)).copy()
    return shared, with_qkbias, with_projbias


_NC_CACHE = {}


def kernel(x, qkv_w, qkv_b, proj_w, proj_b, rel_bias):
    x = np.asarray(x)
    shared, wqk, wpb = _host_prep(x, qkv_w, qkv_b, proj_w, proj_b, rel_bias)

    key = (wqk, wpb)
    if key not in _NC_CACHE:
        _NC_CACHE[key] = _build(NST, wqk, wpb)
    nc = _NC_CACHE[key]

    # feature-major xT per batch element (seq stays the fast axis on chip)
    xt16 = np.ascontiguousarray(
        x.astype(np.float16).transpose(0, 2, 1))          # [B, D, S]
    in_maps = [dict(shared, xt16=xt16[i]) for i in range(B)]
    res = run_bass_kernel_spmd(nc, in_maps, list(range(B)))
    return np.stack([res.results[i]["out"] for i in range(B)], axis=0)


if __name__ == "__main__":
    rng = np.random.default_rng(0)
    x = rng.standard_normal((B, S, D), dtype=np.float32)
    qkv_w = rng.standard_normal((D, 3 * D), dtype=np.float32) / np.sqrt(D)
    proj_w = rng.standard_normal((D, D), dtype=np.float32) / np.sqrt(D)
    out = kernel(x, qkv_w, np.zeros(3 * D, np.float32), proj_w,
                 np.zeros(D, np.float32),
                 rng.standard_normal((2 * W - 1, H), dtype=np.float32) * 0.02)
    print(out.shape, out.dtype)
